# revision 24
# baseline (speedup 1.0000x reference)
"""Causal self-attention with RoPE + attention sinks on 8 Trainium2 NeuronCores.

Sharding: core d handles batch b = d//4 and heads 3*(d%4) .. 3*(d%4)+2
(data parallel on B x tensor parallel on the 12 heads). Each core computes a
partial [T, C] projection output (c_proj contraction over its 3 heads); the
host sums the 4 head-group partials per batch and adds b_proj*rezero.

Device kernel layout choices (v2 — bf16 datapath):
 - All matmul operands are bf16 (1 PE cycle/row, same as f32r, but half the
   DMA/SBUF footprint and 2x DVE); accumulation stays fp32 in PSUM.
 - Phase A is slab-pipelined: xt arrives in 512-column slabs and the QKV
   matmuls for a slab start as soon as it lands, so the PE starts ~4us in
   instead of waiting for the full activation load.
 - qT/kT are computed directly in [d, t] (transposed) layout so scores^T
   tiles [k_tile, q] come out PE-native; softmax exp runs on ACT PSUM->SBUF,
   causal masking via gpsimd affine_select, and the PV matmul needs no
   transposes (lhsT = V tile, rhs = P^T tile).
 - The softmax denominator is a free 65th column of V (ones); the reciprocal
   runs as Ln/Exp on ACT; the q-column broadcast of the reciprocal runs on
   gpsimd partition_broadcast (keeps it off the PE and DVE).
 - RoPE: pair-swap via a permutation-matrix matmul on the PE; the two
   multiply-adds run as fused scalar_tensor_tensor DVE ops which also fold in
   the QKV bias (bias and pair-swapped bias as per-partition scalars).
 - Phase B runs q-chunk-major; the projection (phase C) for a q-chunk is
   emitted as soon as its three heads are normalized, overlapping the
   output projection + DMA with the remaining attention work.
 - yT is packed two heads per 128 partitions, halving projection matmuls.
 - Norm affine, 1/sqrt(hd), and rezero are folded into weights on the host.
"""

import numpy as np
import ml_dtypes

import concourse.bass as bass
import concourse.mybir as mybir
import concourse.tile as tile
from concourse.vector_clock import ScopedClock
from concourse.bass_utils import run_bass_kernel_spmd

F32 = mybir.dt.float32
BF16 = mybir.dt.bfloat16
ALU = mybir.AluOpType
ACTF = mybir.ActivationFunctionType

N_HEAD = 12
N_EMBD = 768
T = 2048
B = 2
HD = 64
HL = 3  # heads per core
NCORES = 8
THETA = 10000.0

KT = 6  # contraction tiles over C=768
NCH = 4  # 512-wide column chunks over T
TTILES = 16  # 128-row tiles over T
VW = 195  # v columns: 3 heads x (64 dims + ones col)

LAST_RESULTS = None  # BassKernelResults of the most recent run (for test.py)


class _TC(tile.TileContext):
    """TileContext whose tail drain splits its sem waits across multiple
    drain instructions; this walrus build rejects >1 wait on an SP Drain."""

    def _drain_and_barrier(self, tick_clock, wait_clock):
        drain_inst = self.nc.sync.drain()
        wait_clock.add_sem_waits(
            drain_inst.ins, ScopedClock({None: tick_clock.global_clock})
        )
        si = drain_inst.ins.sync_info
        if si is not None and len(si.on_wait) > 1:
            waits = list(si.on_wait)
            si.on_wait = waits[:1]
            for w in waits[1:]:
                d2 = self.nc.sync.drain()
                d2.ins.sync_info = mybir.SyncInfo(on_wait=[w], on_update=[])
        self.nc.all_engine_barrier()
        assert self.sems is not None
        popped = self.nc._tile_sem_poison_stack.pop()
        assert popped is self._sem_poison
        self.nc.clear_and_free_semaphores(list(self.sems.allocated().values()))
        self.nc.all_engine_barrier()


def _split_waits(nc, max_waits=1):
    """This walrus build accepts at most one sem-wait per TPB instruction.
    Hoist excess waits of any instruction onto NoOps inserted just before it
    on the same engine (in-order execution keeps semantics identical)."""
    for blk in nc.m.functions[0].blocks:
        new_insts = []
        for inst in blk.instructions:
            si = inst.sync_info
            if si is not None and len(si.on_wait) > max_waits:
                waits = list(si.on_wait)
                extra, keep = waits[:-max_waits], waits[-max_waits:]
                for i in range(0, len(extra), max_waits):
                    nop = mybir.InstNoOp(
                        name=nc.get_next_instruction_name(),
                        engine=inst.engine,
                        ins=[],
                        outs=[],
                        sync_info=mybir.SyncInfo(
                            on_wait=extra[i : i + max_waits], on_update=[]
                        ),
                    )
                    nc.register_instruction(nop)
                    new_insts.append(nop)
                si.on_wait = keep
            new_insts.append(inst)
        blk.instructions[:] = new_insts
    return nc


def build_nc():
    nc = bass.Bass()

    xt_d = nc.dram_tensor("xt", [N_EMBD, T], BF16, kind="ExternalInput")
    wqk_d = nc.dram_tensor("wqk", [N_EMBD, 384], BF16, kind="ExternalInput")
    bqk_d = nc.dram_tensor("bqk", [128, 3], F32, kind="ExternalInput")
    bqksw_d = nc.dram_tensor("bqksw", [128, 3], F32, kind="ExternalInput")
    wv_d = nc.dram_tensor("wv", [N_EMBD, VW], BF16, kind="ExternalInput")
    bv_d = nc.dram_tensor("bv", [1, VW], BF16, kind="ExternalInput")
    wp_d = nc.dram_tensor("wp", [128, 2, N_EMBD], BF16, kind="ExternalInput")
    ct_d = nc.dram_tensor("ct", [128, T], BF16, kind="ExternalInput")
    st_d = nc.dram_tensor("st", [128, T], BF16, kind="ExternalInput")
    perm_d = nc.dram_tensor("perm", [128, 128], BF16, kind="ExternalInput")
    mir_d = nc.dram_tensor("mir", [128, 64], BF16, kind="ExternalInput")
    sink_d = nc.dram_tensor("sinkrow", [1, VW], F32, kind="ExternalInput")
    ones_d = nc.dram_tensor("ones", [1, 512], BF16, kind="ExternalInput")
    yp_d = nc.dram_tensor("yp", [T, N_EMBD], BF16, kind="ExternalOutput")

    with _TC(nc) as tc:
        with (
            tc.tile_pool(name="consts", bufs=1) as consts,
            tc.tile_pool(name="rot", bufs=1) as rotp,
            tc.tile_pool(name="vsb", bufs=1) as vp,
            tc.tile_pool(name="ytp", bufs=1) as ytp,
        ):
            # ---- persistent tiles ----
            ones_sb = consts.tile([1, 512], BF16, tag="ones")
            sink_sb = consts.tile([1, VW], F32, tag="sink")
            wp_sb = consts.tile([128, 2, N_EMBD], BF16, tag="wp")

            # rot1: rotated [k_h0|k_h1]; rot2: rotated [q_h2|k_h2];
            # rot3f: [k_h2 relocated to 0..63 | zeros].
            # zq0/1/2: per-head rotated q zero-padded to 128 partitions so the
            # scores matmuls contract over K=128 (the PE clock governor
            # ignores half-array K=64 matmuls and throttles down).
            rot1 = rotp.tile([128, T], BF16, tag="rot1")
            rot2 = rotp.tile([128, T], BF16, tag="rot2")
            rot3f = rotp.tile([128, T], BF16, tag="rot3f")
            zq = [
                rotp.tile([128, T], BF16, tag=f"zq{i}", name=f"zq{i}")
                for i in range(3)
            ]
            # v tiles [t-tile 128, 3 heads x (64 dims + ones col)]
            v_sb = vp.tile([128, TTILES, VW], BF16, tag="v")
            # yt01: heads 0,1 packed on partitions; yt2: head 2 + zero pad
            yt01 = ytp.tile([128, T], BF16, tag="yt01")
            yt2 = ytp.tile([128, T], BF16, tag="yt2")

            # ================= Phase A: QKV + RoPE + V =================
            # Slab-pipelined: xt lands in 512-column slabs; each slab's QKV
            # matmuls + rope tails + V tiles run while the next slab loads.
            # The rope tail of chunk N is emitted after chunk N+1's QKV
            # matmuls so the PE never stalls on the ACT evacuation of N.
            with (
                tc.tile_pool(name="phA", bufs=1) as phA,
                tc.tile_pool(name="psA", bufs=3, space="PSUM") as psA,
                tc.tile_pool(name="psSw", bufs=3, space="PSUM") as psSw,
                tc.tile_pool(name="psV", bufs=2, space="PSUM") as psV,
                tc.tile_pool(name="evac", bufs=3) as evacp,
                tc.tile_pool(name="tmps", bufs=3) as tmpp,
            ):
                # critical-path loads on the SP queue: wqk, then the xt
                # slabs. Everything else issues from the scalar / gpsimd
                # DGE queues so slab N+1's descriptors are never stuck
                # behind a pile of small-constant issues.
                # Big critical loads split in halves so the transfers spread
                # across more DMA engines (one dma_start saturates only a
                # few rings).
                wqk_sb = phA.tile([128, KT, 384], BF16, tag="wqk")
                wqk_r = wqk_d.rearrange("(ci p) m -> p ci m", p=128)
                nc.sync.dma_start(out=wqk_sb[:, 0:2, :], in_=wqk_r[:, 0:2, :])
                nc.scalar.dma_start(out=wqk_sb[:, 2:4, :], in_=wqk_r[:, 2:4, :])
                nc.sync.dma_start(out=wqk_sb[:, 4:6, :], in_=wqk_r[:, 4:6, :])
                xt_sb = phA.tile([128, KT, T], BF16, tag="xt")
                xt_r = xt_d.rearrange("(ci p) m -> p ci m", p=128)
                ct_sb = phA.tile([128, T], BF16, tag="ct")
                st_sb = phA.tile([128, T], BF16, tag="st")

                for nch in range(NCH):
                    ns = slice(512 * nch, 512 * (nch + 1))
                    nc.sync.dma_start(out=xt_sb[:, 0:2, ns], in_=xt_r[:, 0:2, ns])
                    nc.scalar.dma_start(
                        out=xt_sb[:, 2:4, ns], in_=xt_r[:, 2:4, ns]
                    )
                    nc.sync.dma_start(out=xt_sb[:, 4:6, ns], in_=xt_r[:, 4:6, ns])
                # rope tables + small constants on the gpsimd queue,
                # interleaved with the zero-pad memsets so their transfers
                # don't pile onto the critical xt slabs all at once
                bqk_sb = phA.tile([128, 3], F32, tag="bqk")
                nc.gpsimd.dma_start(out=bqk_sb, in_=bqk_d[:, :])
                bqksw_sb = phA.tile([128, 3], F32, tag="bqksw")
                nc.gpsimd.dma_start(out=bqksw_sb, in_=bqksw_d[:, :])
                perm_sb = phA.tile([128, 128], BF16, tag="perm")
                nc.gpsimd.dma_start(out=perm_sb, in_=perm_d[:, :])
                mir_sb = phA.tile([128, 64], BF16, tag="mir")
                nc.gpsimd.dma_start(out=mir_sb, in_=mir_d[:, :])
                nc.gpsimd.dma_start(out=ones_sb, in_=ones_d[:, :])
                nc.gpsimd.dma_start(out=sink_sb, in_=sink_d[:, :])
                nc.gpsimd.dma_start(out=ct_sb[:, 0:512], in_=ct_d[:, 0:512])
                nc.gpsimd.dma_start(out=st_sb[:, 0:512], in_=st_d[:, 0:512])
                nc.gpsimd.memset(zq[0][64:128, :], 0.0)
                nc.gpsimd.dma_start(out=ct_sb[:, 512:1024], in_=ct_d[:, 512:1024])
                nc.gpsimd.dma_start(out=st_sb[:, 512:1024], in_=st_d[:, 512:1024])
                nc.gpsimd.memset(zq[1][0:64, :], 0.0)
                nc.gpsimd.dma_start(out=ct_sb[:, 1024:2048], in_=ct_d[:, 1024:2048])
                nc.gpsimd.dma_start(out=st_sb[:, 1024:2048], in_=st_d[:, 1024:2048])
                nc.gpsimd.memset(zq[2][64:128, :], 0.0)
                nc.gpsimd.memset(rot3f[64:128, :], 0.0)
                nc.gpsimd.memset(yt2[64:128, :], 0.0)
                # weights for V / projection on the scalar queue
                wv_sb = phA.tile([128, KT, VW], BF16, tag="wv")
                nc.scalar.dma_start(
                    out=wv_sb, in_=wv_d.rearrange("(ci p) m -> p ci m", p=128)
                )
                bv_sb = phA.tile([1, VW], BF16, tag="bv")
                nc.scalar.dma_start(out=bv_sb, in_=bv_d[:, :])
                nc.scalar.dma_start(out=wp_sb, in_=wp_d[:, :, :])

                def rope_tail(mc, nch, qk_ps, qk_sb):
                    ns = slice(512 * nch, 512 * (nch + 1))
                    sw_ps = psSw.tile(
                        [128, 512], F32, tag="sw", bufs=2, name=f"sw{mc}_{nch}"
                    )
                    nc.tensor.matmul(sw_ps, perm_sb, qk_sb, start=True, stop=True)
                    sw_sb = tmpp.tile(
                        [128, 512], BF16, tag="swsb", name=f"swsb{mc}_{nch}"
                    )
                    nc.scalar.copy(out=sw_sb, in_=sw_ps)
                    tmp1 = tmpp.tile([128, 512], BF16, tag="t1", name=f"t1_{mc}_{nch}")
                    nc.vector.scalar_tensor_tensor(
                        out=tmp1,
                        in0=qk_sb,
                        scalar=bqk_sb[:, mc : mc + 1],
                        in1=ct_sb[:, ns],
                        op0=ALU.add,
                        op1=ALU.mult,
                    )
                    tmp2 = tmpp.tile([128, 512], BF16, tag="t2", name=f"t2_{mc}_{nch}")
                    nc.vector.scalar_tensor_tensor(
                        out=tmp2,
                        in0=sw_sb,
                        scalar=bqksw_sb[:, mc : mc + 1],
                        in1=st_sb[:, ns],
                        op0=ALU.add,
                        op1=ALU.mult,
                    )
                    if mc == 0:
                        nc.vector.tensor_add(
                            out=zq[0][0:64, ns], in0=tmp1[0:64, :], in1=tmp2[0:64, :]
                        )
                        nc.vector.tensor_add(
                            out=zq[1][64:128, ns],
                            in0=tmp1[64:128, :],
                            in1=tmp2[64:128, :],
                        )
                    elif mc == 1:
                        nc.vector.tensor_add(out=rot1[:, ns], in0=tmp1, in1=tmp2)
                    else:
                        nc.vector.tensor_add(out=rot2[:, ns], in0=tmp1, in1=tmp2)
                        m_ps = psSw.tile(
                            [64, 512], F32, tag="mir", bufs=1, name=f"m{nch}"
                        )
                        nc.tensor.matmul(
                            m_ps, mir_sb, rot2[:, ns], start=True, stop=True
                        )
                        nc.scalar.copy(out=rot3f[0:64, ns], in_=m_ps)
                        nc.gpsimd.tensor_copy(out=zq[2][0:64, ns], in_=rot2[0:64, ns])

                pending = None
                for nch in range(NCH):
                    ns = slice(512 * nch, 512 * (nch + 1))
                    for mc in range(3):
                        qk_ps = psA.tile(
                            [128, 512], F32, tag="qk", bufs=3, name=f"qk{mc}_{nch}"
                        )
                        for ci in range(KT):
                            nc.tensor.matmul(
                                qk_ps,
                                wqk_sb[:, ci, 128 * mc : 128 * (mc + 1)],
                                xt_sb[:, ci, ns],
                                start=(ci == 0),
                                stop=(ci == KT - 1),
                            )
                        qk_sb = evacp.tile(
                            [128, 512], BF16, tag="qkev", name=f"qkev{mc}_{nch}"
                        )
                        nc.scalar.copy(out=qk_sb, in_=qk_ps)
                        if pending is not None:
                            rope_tail(*pending)
                        pending = (mc, nch, qk_ps, qk_sb)
                    # V tiles for this slab overlap the rope tails on ACT/DVE
                    for ti in range(4 * nch, 4 * nch + 4):
                        v_ps = psV.tile([128, VW], F32, tag="vps", name=f"vps{ti}")
                        for ci in range(KT):
                            nc.tensor.matmul(
                                v_ps,
                                xt_sb[:, ci, 128 * ti : 128 * (ti + 1)],
                                wv_sb[:, ci, :],
                                start=(ci == 0),
                                stop=False,
                            )
                        nc.tensor.matmul(
                            v_ps,
                            ones_sb[0:1, 0:128],
                            bv_sb,
                            start=False,
                            stop=True,
                        )
                        nc.scalar.copy(out=v_sb[:, ti, :], in_=v_ps)
                rope_tail(*pending)

            # ============ Phase B: attention + interleaved projection ============
            # q-chunk-major: for each 512-wide q chunk, all three heads run
            # scores->exp, then the PV accumulation of iteration N runs after
            # iteration N+1's score matmuls (so the PE never stalls on the
            # Exp/affine_select of the last diagonal tile), the normalization
            # tail of N is likewise flushed one iteration late, and the
            # output projection of a finished q chunk follows its last
            # normalization flush.
            qsl = [zq[0], zq[1], zq[2]]
            ksl = [rot1, rot1, rot3f]
            # norm_tail output slot per head: (tile, row offset)
            ydst = [(yt01, 0), (yt01, 64), (yt2, 0)]
            with (
                tc.tile_pool(name="psS", bufs=2, space="PSUM") as psS,
                tc.tile_pool(name="psY", bufs=2, space="PSUM") as psY,
                tc.tile_pool(name="psP", bufs=2, space="PSUM") as psP,
                tc.tile_pool(name="pt", bufs=18) as ptp,
                tc.tile_pool(name="smax", bufs=3) as smaxp,
                tc.tile_pool(name="pout", bufs=4) as poutp,
                tc.tile_pool(name="rdram", bufs=3, space="DRAM") as rdp,
            ):

                def norm_head(hp, qc, y_ps, fast=False):
                    # r = exp(-ln(denom+sink)) on ACT: Ln/Exp/Copy share one
                    # activation table; the reciprocal stays off the DVE and
                    # off the PE critical path. The q-column broadcast runs
                    # as a DRAM round-trip (stride-0 partition read) on the
                    # idle gpsimd DMA queue — no PE matmul, no PSUM bank.
                    # fast=True (drain tail): rank-1 PE matmul into the
                    # unused partitions 64..127 instead — shorter latency.
                    lnr = smaxp.tile([1, 512], F32, tag="lnr", name=f"ln{hp}_{qc}")
                    nc.scalar.activation(
                        out=lnr,
                        in_=y_ps[64:65, :],
                        func=ACTF.Ln,
                        bias=sink_sb[0:1, 65 * hp + 64 : 65 * hp + 65],
                    )
                    r_sb = smaxp.tile([1, 512], BF16, tag="r", name=f"r{hp}_{qc}")
                    nc.scalar.activation(out=r_sb, in_=lnr, func=ACTF.Exp, scale=-1.0)
                    yu = smaxp.tile([64, 512], BF16, tag="yu", name=f"yu{hp}_{qc}")
                    nc.vector.tensor_copy(out=yu, in_=y_ps[0:64, :])
                    if fast:
                        nc.tensor.matmul(
                            y_ps[64:128, :],
                            ones_sb[0:1, 0:64],
                            r_sb,
                            start=True,
                            stop=True,
                        )
                        return (yu, y_ps[64:128, :])
                    scr = rdp.tile([1, 512], BF16, tag="scr", name=f"scr{hp}_{qc}")
                    nc.gpsimd.dma_start(out=scr, in_=r_sb)
                    rb_sb = smaxp.tile([64, 512], BF16, tag="rb", name=f"rb{hp}_{qc}")
                    nc.gpsimd.dma_start(
                        out=rb_sb,
                        in_=bass.AP(
                            tensor=scr.tensor, offset=scr.offset, ap=[[0, 64], [1, 512]]
                        ),
                    )
                    return (yu, rb_sb)

                def norm_finish(hp, qc, yu_rb):
                    yu, rb = yu_rb
                    dst, ro = ydst[hp]
                    nc.vector.tensor_mul(
                        out=dst[ro : ro + 64, 512 * qc : 512 * (qc + 1)],
                        in0=yu,
                        in1=rb,
                    )

                def emit_proj(qc):
                    for ti in range(4 * qc, 4 * qc + 4):
                        for nn in range(2):
                            p_ps = psP.tile(
                                [128, 384], F32, tag="p", name=f"p{ti}_{nn}"
                            )
                            nc.tensor.matmul(
                                p_ps,
                                yt01[:, 128 * ti : 128 * (ti + 1)],
                                wp_sb[:, 0, 384 * nn : 384 * (nn + 1)],
                                start=True,
                                stop=False,
                            )
                            nc.tensor.matmul(
                                p_ps,
                                yt2[:, 128 * ti : 128 * (ti + 1)],
                                wp_sb[:, 1, 384 * nn : 384 * (nn + 1)],
                                start=False,
                                stop=True,
                            )
                            p_sb = poutp.tile(
                                [128, 384], BF16, tag="psb", name=f"psb{ti}_{nn}"
                            )
                            if (ti + nn) % 2 == 0:
                                nc.vector.tensor_copy(out=p_sb, in_=p_ps)
                            else:
                                nc.scalar.copy(out=p_sb, in_=p_ps)
                            nc.sync.dma_start(
                                out=yp_d[
                                    128 * ti : 128 * (ti + 1),
                                    384 * nn : 384 * (nn + 1),
                                ],
                                in_=p_sb,
                            )

                def score_units(qc, hp, pts):
                    """Yield per-pair emitters: each emits 2 score matmuls +
                    1 Exp (+ affine_selects for diagonal pairs) and appends
                    to pts. Interleaving these with the previous iteration's
                    PV matmuls keeps ACT fed during the PV chain."""
                    qt = qsl[hp]
                    kt_ = ksl[hp]
                    nki = 4 * qc + 4
                    nfull = 4 * qc  # k-tiles with no causal masking

                    def full_pair(kp):
                        st2 = psS.tile(
                            [128, 1024],
                            F32,
                            tag="st2",
                            bufs=2,
                            name=f"st2_{hp}_{qc}_{kp}",
                        )
                        pt2 = ptp.tile(
                            [128, 1024],
                            BF16,
                            tag="pt",
                            name=f"pt_{hp}_{qc}_{kp}",
                            bufs=14,
                        )
                        for j in range(2):
                            ki = kp + j
                            nc.tensor.matmul(
                                st2[:, 512 * j : 512 * (j + 1)],
                                kt_[:, 128 * ki : 128 * (ki + 1)],
                                qt[:, 512 * qc : 512 * (qc + 1)],
                                start=True,
                                stop=True,
                            )
                        nc.scalar.activation(out=pt2, in_=st2, func=ACTF.Exp)
                        pts.append((pt2[:, 0:512], 0))
                        pts.append((pt2[:, 512:1024], 0))

                    def masked_pair(kp):
                        lefts = [
                            max(0, 128 * (kp + j) - 512 * qc) for j in range(2)
                        ]
                        widths = [512 - lf for lf in lefts]
                        tot = widths[0] + widths[1]
                        st_ps = psS.tile(
                            [128, 1024],
                            F32,
                            tag="st2",
                            bufs=2,
                            name=f"st{hp}_{qc}_{kp}",
                        )
                        ptm = ptp.tile(
                            [128, 1024],
                            BF16,
                            tag="ptm",
                            name=f"ptm_{hp}_{qc}_{kp}",
                            bufs=4,
                        )
                        off = 0
                        sub = []
                        for j in range(2):
                            ki = kp + j
                            nc.tensor.matmul(
                                st_ps[:, off : off + widths[j]],
                                kt_[:, 128 * ki : 128 * (ki + 1)],
                                qt[:, 512 * qc + lefts[j] : 512 * (qc + 1)],
                                start=True,
                                stop=True,
                            )
                            sub.append((off, widths[j], lefts[j]))
                            off += widths[j]
                        nc.scalar.activation(
                            out=ptm[:, :tot], in_=st_ps[:, :tot], func=ACTF.Exp
                        )
                        for j in range(2):
                            off_j, w_j, lf_j = sub[j]
                            nc.gpsimd.affine_select(
                                out=ptm[:, off_j : off_j + 128],
                                in_=ptm[:, off_j : off_j + 128],
                                pattern=[[1, 128]],
                                base=0,
                                channel_multiplier=-1,
                                compare_op=ALU.is_ge,
                                fill=0.0,
                            )
                            pts.append((ptm[:, off_j : off_j + w_j], lf_j))

                    for kp in range(0, nfull, 2):
                        yield lambda kp=kp: full_pair(kp)
                    for kp in range(nfull, nki, 2):
                        yield lambda kp=kp: masked_pair(kp)

                def pv_units(qc, hp, pts, y_ps):
                    nki = 4 * qc + 4

                    def one(ki):
                        rhs_ap, left = pts[ki]
                        nc.tensor.matmul(
                            y_ps[0:65, left:],
                            v_sb[:, ki, 65 * hp : 65 * hp + 65],
                            rhs_ap,
                            start=(ki == 0),
                            stop=(ki == nki - 1),
                            skip_group_check=True,
                        )

                    for ki in range(nki):
                        yield lambda ki=ki: one(ki)

                iters = [(qc, hp) for qc in range(NCH) for hp in range(3)]
                prev = None  # (qc, hp, pts) awaiting PV
                pending_fin = None

                def tail_stage(pqc, php, y_ps, fast=False):
                    # norm head for the PV that just finished; flush the
                    # finish + projection of the one before it
                    nonlocal pending_fin
                    yu_rb = norm_head(php, pqc, y_ps, fast=fast)
                    if pending_fin is not None:
                        norm_finish(*pending_fin)
                        if pending_fin[0] == 2:
                            emit_proj(pending_fin[1])
                    pending_fin = (php, pqc, yu_rb)

                last_iter = iters[-1]
                for qc, hp in iters:
                    pts = []
                    sus = list(score_units(qc, hp, pts))
                    if prev is not None:
                        pqc, php, ppts = prev
                        y_ps = psY.tile(
                            [128, 512], F32, tag="y", name=f"y{php}_{pqc}"
                        )
                        pvs = list(pv_units(pqc, php, ppts, y_ps))
                    else:
                        y_ps = None
                        pvs = []
                    # interleave: 1 score pair then ~2 PV matmuls so the
                    # Exp queue on ACT never runs dry during the PV chain
                    nsu, npv = len(sus), len(pvs)
                    pi = 0
                    for si, su in enumerate(sus):
                        su()
                        target = (si + 1) * npv // nsu if nsu else npv
                        while pi < target:
                            pvs[pi]()
                            pi += 1
                    while pi < npv:
                        pvs[pi]()
                        pi += 1
                    if prev is not None:
                        tail_stage(prev[0], prev[1], y_ps, fast=(qc, hp) == last_iter)
                    prev = (qc, hp, pts)
                # drain the pipeline
                pqc, php, ppts = prev
                y_ps = psY.tile([128, 512], F32, tag="y", name=f"y{php}_{pqc}")
                for pv in pv_units(pqc, php, ppts, y_ps):
                    pv()
                tail_stage(pqc, php, y_ps, fast=True)
                norm_finish(*pending_fin)
                emit_proj(NCH - 1)

    _split_waits(nc)
    return nc


_NC_CACHE = {}


def _get_nc():
    if "nc" not in _NC_CACHE:
        _NC_CACHE["nc"] = build_nc()
    return _NC_CACHE["nc"]


def _prep_core_inputs(inputs):
    """Host-side sharding: fold norm/scale/rezero into weights, build the
    per-core input maps."""
    bf16 = ml_dtypes.bfloat16
    x = np.asarray(inputs["x"], np.float32)
    ns_ = np.asarray(inputs["norm_scale"], np.float32)
    nb_ = np.asarray(inputs["norm_bias"], np.float32)
    Wa = np.asarray(inputs["W_attn"], np.float32)
    ba = np.asarray(inputs["b_attn"], np.float32)
    Wp = np.asarray(inputs["W_proj"], np.float32)
    sinks = np.asarray(inputs["sinks"], np.float32)
    rz = float(np.asarray(inputs["rezero"], np.float32).reshape(()))

    C = N_EMBD
    W = (ns_[:, None] * Wa).astype(np.float32)
    beff = (nb_.astype(np.float64) @ Wa.astype(np.float64) + ba).astype(np.float32)
    scale = 1.0 / np.sqrt(np.float32(HD))

    # RoPE tables, interleaved-row layout (64-row periodic)
    freqs = 1.0 / THETA ** (np.arange(0, HD, 2, dtype=np.float64) / HD)  # [32]
    tpos = np.arange(T, dtype=np.float64)
    ang = np.outer(tpos, freqs)  # [T, 32]
    cos_t = np.cos(ang).T  # [32, T]
    sin_t = np.sin(ang).T
    ct = np.empty((128, T), np.float64)
    st = np.empty((128, T), np.float64)
    for blk in range(2):
        r0 = 64 * blk
        ct[r0 + 0 : r0 + 64 : 2] = cos_t
        ct[r0 + 1 : r0 + 64 : 2] = cos_t
        st[r0 + 0 : r0 + 64 : 2] = -sin_t
        st[r0 + 1 : r0 + 64 : 2] = sin_t
    ct = ct.astype(bf16)
    st = st.astype(bf16)

    perm = np.zeros((128, 128), np.float32)
    for i in range(64):
        perm[2 * i, 2 * i + 1] = 1.0
        perm[2 * i + 1, 2 * i] = 1.0
    mir = np.zeros((128, 64), np.float32)
    for i in range(64):
        mir[64 + i, i] = 1.0
    ones = np.ones((1, 512), np.float32)
    swap_idx = np.arange(128)
    swap_idx = swap_idx + 1 - 2 * (swap_idx % 2)  # pairwise swap

    in_maps = []
    for d in range(NCORES):
        b = d // 4
        g = d % 4
        heads = [3 * g + j for j in range(HL)]

        wqk = np.empty((C, 384), np.float32)
        bqk = np.empty((128, 3), np.float32)
        # c0 = [q_h0 | q_h1], c1 = [k_h0 | k_h1], c2 = [q_h2 | k_h2]
        h0, h1, h2 = heads
        wqk[:, 0:64] = W[:, 64 * h0 : 64 * h0 + 64] * scale
        wqk[:, 64:128] = W[:, 64 * h1 : 64 * h1 + 64] * scale
        wqk[:, 128:192] = W[:, C + 64 * h0 : C + 64 * h0 + 64]
        wqk[:, 192:256] = W[:, C + 64 * h1 : C + 64 * h1 + 64]
        wqk[:, 256:320] = W[:, 64 * h2 : 64 * h2 + 64] * scale
        wqk[:, 320:384] = W[:, C + 64 * h2 : C + 64 * h2 + 64]
        bqk[0:64, 0] = beff[64 * h0 : 64 * h0 + 64] * scale
        bqk[64:128, 0] = beff[64 * h1 : 64 * h1 + 64] * scale
        bqk[0:64, 1] = beff[C + 64 * h0 : C + 64 * h0 + 64]
        bqk[64:128, 1] = beff[C + 64 * h1 : C + 64 * h1 + 64]
        bqk[0:64, 2] = beff[64 * h2 : 64 * h2 + 64] * scale
        bqk[64:128, 2] = beff[C + 64 * h2 : C + 64 * h2 + 64]
        bqksw = bqk[swap_idx, :].copy()

        wv = np.zeros((C, VW), np.float32)
        bv = np.zeros((1, VW), np.float32)
        for j, h in enumerate(heads):
            wv[:, 65 * j : 65 * j + 64] = W[:, 2 * C + 64 * h : 2 * C + 64 * h + 64]
            bv[0, 65 * j : 65 * j + 64] = beff[2 * C + 64 * h : 2 * C + 64 * h + 64]
            bv[0, 65 * j + 64] = 1.0

        # projection weights: heads 0,1 packed on partitions; head 2 + pad
        wp = np.zeros((128, 2, C), np.float32)
        wp[0:64, 0, :] = Wp[64 * h0 : 64 * h0 + 64, :] * rz
        wp[64:128, 0, :] = Wp[64 * h1 : 64 * h1 + 64, :] * rz
        wp[0:64, 1, :] = Wp[64 * h2 : 64 * h2 + 64, :] * rz

        sinkrow = np.zeros((1, VW), np.float32)
        for j, h in enumerate(heads):
            sinkrow[0, 65 * j + 64] = np.exp(np.float64(sinks[h]))

        in_maps.append(
            {
                "xt": np.ascontiguousarray(x[b].T).astype(bf16),
                "wqk": wqk.astype(bf16),
                "bqk": bqk,
                "bqksw": bqksw,
                "wv": wv.astype(bf16),
                "bv": bv.astype(bf16),
                "wp": wp.astype(bf16),
                "ct": ct,
                "st": st,
                "perm": perm.astype(bf16),
                "mir": mir.astype(bf16),
                "sinkrow": sinkrow,
                "ones": ones.astype(bf16),
            }
        )

    bias_out = (np.asarray(inputs["b_proj"], np.float32) * rz).astype(np.float32)
    return in_maps, bias_out


def kernel(**inputs):
    global LAST_RESULTS
    nc = _get_nc()
    in_maps, bias_out = _prep_core_inputs(inputs)
    res = None
    last_exc = None
    for attempt in range(3):
        try:
            res = run_bass_kernel_spmd(nc, in_maps, core_ids=list(range(NCORES)))
            break
        except Exception as e:  # transient NRT_EXEC_UNIT_UNRECOVERABLE etc.
            last_exc = e
            import time as _time

            _time.sleep(2.0)
    if res is None:
        raise last_exc
    LAST_RESULTS = res
    y = np.zeros((B, T, N_EMBD), np.float32)
    for d in range(NCORES):
        y[d // 4] += np.asarray(res.results[d]["yp"], dtype=np.float32)
    y += bias_out[None, None, :]
    return y


# revision 25
# speedup vs baseline: 1.0138x; 1.0138x over previous
"""Causal self-attention with RoPE + attention sinks on 8 Trainium2 NeuronCores.

Sharding: core d handles batch b = d//4 and heads 3*(d%4) .. 3*(d%4)+2
(data parallel on B x tensor parallel on the 12 heads). Each core computes a
partial [T, C] projection output (c_proj contraction over its 3 heads); the
host sums the 4 head-group partials per batch and adds b_proj*rezero.

Device kernel layout choices (bf16 datapath):
 - All matmul operands are bf16 (1 PE cycle/row, same as f32r, but half the
   DMA/SBUF footprint and 2x DVE); accumulation stays fp32 in PSUM.
 - Phase A is slab-pipelined: xt arrives in 512-column slabs and the QKV
   matmuls for a slab start as soon as it lands, so the PE starts ~4us in
   instead of waiting for the full activation load.
 - qT/kT are computed directly in [d, t] (transposed) layout so scores^T
   tiles [k_tile, q] come out PE-native; softmax exp runs on ACT PSUM->SBUF,
   causal masking via gpsimd affine_select, and the PV matmul needs no
   transposes (lhsT = V tile, rhs = P^T tile).
 - The softmax denominator is a free 65th column of V (ones); the reciprocal
   runs as Ln/Exp on ACT; the q-column broadcast of the reciprocal runs as a
   DRAM round-trip DMA (stride-0 partition read) on the gpsimd queue, with a
   rank-1 PE matmul fallback for the drain tail.
 - RoPE: pair-swap via a permutation-matrix matmul on the PE; the two
   multiply-adds run as fused scalar_tensor_tensor DVE ops which also fold in
   the QKV bias (bias and pair-swapped bias as per-partition scalars).
 - Phase B runs q-chunk-major with software pipelining: the PV chain of
   iteration N is interleaved 2:1 between iteration N+1's score pairs so the
   ACT Exp queue never runs dry; the projection for a q-chunk is emitted as
   soon as its three heads are normalized, overlapping projection + output
   DMA with the remaining attention work.
 - yT is packed two heads per 128 partitions, halving projection matmuls.
 - Norm affine, 1/sqrt(hd), and rezero are folded into weights on the host.
"""

import numpy as np
import ml_dtypes

import concourse.bass as bass
import concourse.mybir as mybir
import concourse.tile as tile
from concourse.vector_clock import ScopedClock
from concourse.bass_utils import run_bass_kernel_spmd

F32 = mybir.dt.float32
BF16 = mybir.dt.bfloat16
ALU = mybir.AluOpType
ACTF = mybir.ActivationFunctionType

N_HEAD = 12
N_EMBD = 768
T = 2048
B = 2
HD = 64
HL = 3  # heads per core
NCORES = 8
THETA = 10000.0

KT = 6  # contraction tiles over C=768
NCH = 4  # 512-wide column chunks over T
TTILES = 16  # 128-row tiles over T
VW = 195  # v columns: 3 heads x (64 dims + ones col)

LAST_RESULTS = None  # BassKernelResults of the most recent run (for test.py)


class _TC(tile.TileContext):
    """TileContext whose tail drain splits its sem waits across multiple
    drain instructions; this walrus build rejects >1 wait on an SP Drain."""

    def _drain_and_barrier(self, tick_clock, wait_clock):
        drain_inst = self.nc.sync.drain()
        wait_clock.add_sem_waits(
            drain_inst.ins, ScopedClock({None: tick_clock.global_clock})
        )
        si = drain_inst.ins.sync_info
        if si is not None and len(si.on_wait) > 1:
            waits = list(si.on_wait)
            si.on_wait = waits[:1]
            for w in waits[1:]:
                d2 = self.nc.sync.drain()
                d2.ins.sync_info = mybir.SyncInfo(on_wait=[w], on_update=[])
        self.nc.all_engine_barrier()
        assert self.sems is not None
        popped = self.nc._tile_sem_poison_stack.pop()
        assert popped is self._sem_poison
        self.nc.clear_and_free_semaphores(list(self.sems.allocated().values()))
        self.nc.all_engine_barrier()


def _split_waits(nc, max_waits=1):
    """This walrus build accepts at most one sem-wait per TPB instruction.
    Hoist excess waits of any instruction onto NoOps inserted just before it
    on the same engine (in-order execution keeps semantics identical)."""
    for blk in nc.m.functions[0].blocks:
        new_insts = []
        for inst in blk.instructions:
            si = inst.sync_info
            if si is not None and len(si.on_wait) > max_waits:
                waits = list(si.on_wait)
                extra, keep = waits[:-max_waits], waits[-max_waits:]
                for i in range(0, len(extra), max_waits):
                    nop = mybir.InstNoOp(
                        name=nc.get_next_instruction_name(),
                        engine=inst.engine,
                        ins=[],
                        outs=[],
                        sync_info=mybir.SyncInfo(
                            on_wait=extra[i : i + max_waits], on_update=[]
                        ),
                    )
                    nc.register_instruction(nop)
                    new_insts.append(nop)
                si.on_wait = keep
            new_insts.append(inst)
        blk.instructions[:] = new_insts
    return nc


def build_nc():
    nc = bass.Bass()

    xt_d = nc.dram_tensor("xt", [N_EMBD, T], BF16, kind="ExternalInput")
    wqk_d = nc.dram_tensor("wqk", [N_EMBD, 384], BF16, kind="ExternalInput")
    bqk_d = nc.dram_tensor("bqk", [128, 3], F32, kind="ExternalInput")
    bqksw_d = nc.dram_tensor("bqksw", [128, 3], F32, kind="ExternalInput")
    wv_d = nc.dram_tensor("wv", [N_EMBD, VW], BF16, kind="ExternalInput")
    bv_d = nc.dram_tensor("bv", [1, VW], BF16, kind="ExternalInput")
    wp_d = nc.dram_tensor("wp", [128, 2, N_EMBD], BF16, kind="ExternalInput")
    ct_d = nc.dram_tensor("ct", [128, T], BF16, kind="ExternalInput")
    st_d = nc.dram_tensor("st", [128, T], BF16, kind="ExternalInput")
    perm_d = nc.dram_tensor("perm", [128, 128], BF16, kind="ExternalInput")
    mir_d = nc.dram_tensor("mir", [128, 64], BF16, kind="ExternalInput")
    sink_d = nc.dram_tensor("sinkrow", [1, VW], F32, kind="ExternalInput")
    ones_d = nc.dram_tensor("ones", [1, 512], BF16, kind="ExternalInput")
    yp_d = nc.dram_tensor("yp", [T, N_EMBD], BF16, kind="ExternalOutput")

    with _TC(nc) as tc:
        with (
            tc.tile_pool(name="consts", bufs=1) as consts,
            tc.tile_pool(name="rot", bufs=1) as rotp,
            tc.tile_pool(name="vsb", bufs=1) as vp,
            tc.tile_pool(name="ytp", bufs=1) as ytp,
        ):
            # ---- persistent tiles ----
            ones_sb = consts.tile([1, 512], BF16, tag="ones")
            sink_sb = consts.tile([1, VW], F32, tag="sink")
            wp_sb = consts.tile([128, 2, N_EMBD], BF16, tag="wp")

            # rot1: rotated [k_h0|k_h1]; rot2: rotated [q_h2|k_h2];
            # rot3f: [k_h2 relocated to 0..63 | zeros].
            # zq0/1/2: per-head rotated q zero-padded to 128 partitions so the
            # scores matmuls contract over K=128 (the PE clock governor
            # ignores half-array K=64 matmuls and throttles down).
            rot1 = rotp.tile([128, T], BF16, tag="rot1")
            rot2 = rotp.tile([128, T], BF16, tag="rot2")
            rot3f = rotp.tile([128, T], BF16, tag="rot3f")
            zq = [
                rotp.tile([128, T], BF16, tag=f"zq{i}", name=f"zq{i}")
                for i in range(3)
            ]
            # v tiles [t-tile 128, 3 heads x (64 dims + ones col)]
            v_sb = vp.tile([128, TTILES, VW], BF16, tag="v")
            # yt01: heads 0,1 packed on partitions; yt2: head 2 + zero pad
            yt01 = ytp.tile([128, T], BF16, tag="yt01")
            yt2 = ytp.tile([128, T], BF16, tag="yt2")

            # ================= Phase A: QKV + RoPE + V =================
            # Slab-pipelined: xt lands in 512-column slabs; each slab's QKV
            # matmuls + rope tails + V tiles run while the next slab loads.
            # The rope tail of chunk N is emitted after chunk N+1's QKV
            # matmuls so the PE never stalls on the ACT evacuation of N.
            with (
                tc.tile_pool(name="phA", bufs=1) as phA,
                tc.tile_pool(name="psA", bufs=3, space="PSUM") as psA,
                tc.tile_pool(name="psSw", bufs=3, space="PSUM") as psSw,
                tc.tile_pool(name="psV", bufs=2, space="PSUM") as psV,
                tc.tile_pool(name="evac", bufs=3) as evacp,
                tc.tile_pool(name="tmps", bufs=3) as tmpp,
            ):
                # critical-path loads on the SP queue: wqk, then the xt
                # slabs. Everything else issues from the scalar / gpsimd
                # DGE queues so slab N+1's descriptors are never stuck
                # behind a pile of small-constant issues.
                # Big critical loads split in halves so the transfers spread
                # across more DMA engines (one dma_start saturates only a
                # few rings).
                wqk_sb = phA.tile([128, KT, 384], BF16, tag="wqk")
                wqk_r = wqk_d.rearrange("(ci p) m -> p ci m", p=128)
                nc.sync.dma_start(out=wqk_sb[:, 0:2, :], in_=wqk_r[:, 0:2, :])
                nc.scalar.dma_start(out=wqk_sb[:, 2:4, :], in_=wqk_r[:, 2:4, :])
                nc.sync.dma_start(out=wqk_sb[:, 4:6, :], in_=wqk_r[:, 4:6, :])
                xt_sb = phA.tile([128, KT, T], BF16, tag="xt")
                xt_r = xt_d.rearrange("(ci p) m -> p ci m", p=128)
                ct_sb = phA.tile([128, T], BF16, tag="ct")
                st_sb = phA.tile([128, T], BF16, tag="st")

                for nch in range(NCH):
                    ns = slice(512 * nch, 512 * (nch + 1))
                    nc.sync.dma_start(out=xt_sb[:, 0:2, ns], in_=xt_r[:, 0:2, ns])
                    nc.scalar.dma_start(
                        out=xt_sb[:, 2:4, ns], in_=xt_r[:, 2:4, ns]
                    )
                    nc.sync.dma_start(out=xt_sb[:, 4:6, ns], in_=xt_r[:, 4:6, ns])
                # rope tables + small constants on the gpsimd queue,
                # interleaved with the zero-pad memsets so their transfers
                # don't pile onto the critical xt slabs all at once
                bqk_sb = phA.tile([128, 3], F32, tag="bqk")
                nc.gpsimd.dma_start(out=bqk_sb, in_=bqk_d[:, :])
                bqksw_sb = phA.tile([128, 3], F32, tag="bqksw")
                nc.gpsimd.dma_start(out=bqksw_sb, in_=bqksw_d[:, :])
                perm_sb = phA.tile([128, 128], BF16, tag="perm")
                nc.gpsimd.dma_start(out=perm_sb, in_=perm_d[:, :])
                mir_sb = phA.tile([128, 64], BF16, tag="mir")
                nc.gpsimd.dma_start(out=mir_sb, in_=mir_d[:, :])
                nc.gpsimd.dma_start(out=ones_sb, in_=ones_d[:, :])
                nc.gpsimd.dma_start(out=sink_sb, in_=sink_d[:, :])
                nc.gpsimd.dma_start(out=ct_sb[:, 0:512], in_=ct_d[:, 0:512])
                nc.gpsimd.dma_start(out=st_sb[:, 0:512], in_=st_d[:, 0:512])
                nc.gpsimd.memset(zq[0][64:128, :], 0.0)
                nc.gpsimd.dma_start(out=ct_sb[:, 512:1024], in_=ct_d[:, 512:1024])
                nc.gpsimd.dma_start(out=st_sb[:, 512:1024], in_=st_d[:, 512:1024])
                nc.gpsimd.memset(zq[1][0:64, :], 0.0)
                nc.gpsimd.dma_start(out=ct_sb[:, 1024:2048], in_=ct_d[:, 1024:2048])
                nc.gpsimd.dma_start(out=st_sb[:, 1024:2048], in_=st_d[:, 1024:2048])
                nc.gpsimd.memset(zq[2][64:128, :], 0.0)
                nc.gpsimd.memset(rot3f[64:128, :], 0.0)
                nc.gpsimd.memset(yt2[64:128, :], 0.0)
                # weights for V / projection on the scalar queue
                wv_sb = phA.tile([128, KT, VW], BF16, tag="wv")
                nc.scalar.dma_start(
                    out=wv_sb, in_=wv_d.rearrange("(ci p) m -> p ci m", p=128)
                )
                bv_sb = phA.tile([1, VW], BF16, tag="bv")
                nc.scalar.dma_start(out=bv_sb, in_=bv_d[:, :])
                nc.scalar.dma_start(out=wp_sb, in_=wp_d[:, :, :])

                def rope_tail(mc, nch, qk_ps, qk_sb):
                    ns = slice(512 * nch, 512 * (nch + 1))
                    sw_ps = psSw.tile(
                        [128, 512], F32, tag="sw", bufs=2, name=f"sw{mc}_{nch}"
                    )
                    nc.tensor.matmul(sw_ps, perm_sb, qk_sb, start=True, stop=True)
                    sw_sb = tmpp.tile(
                        [128, 512], BF16, tag="swsb", name=f"swsb{mc}_{nch}"
                    )
                    nc.scalar.copy(out=sw_sb, in_=sw_ps)
                    tmp1 = tmpp.tile([128, 512], BF16, tag="t1", name=f"t1_{mc}_{nch}")
                    nc.vector.scalar_tensor_tensor(
                        out=tmp1,
                        in0=qk_sb,
                        scalar=bqk_sb[:, mc : mc + 1],
                        in1=ct_sb[:, ns],
                        op0=ALU.add,
                        op1=ALU.mult,
                    )
                    tmp2 = tmpp.tile([128, 512], BF16, tag="t2", name=f"t2_{mc}_{nch}")
                    nc.vector.scalar_tensor_tensor(
                        out=tmp2,
                        in0=sw_sb,
                        scalar=bqksw_sb[:, mc : mc + 1],
                        in1=st_sb[:, ns],
                        op0=ALU.add,
                        op1=ALU.mult,
                    )
                    if mc == 0:
                        nc.vector.tensor_add(
                            out=zq[0][0:64, ns], in0=tmp1[0:64, :], in1=tmp2[0:64, :]
                        )
                        nc.vector.tensor_add(
                            out=zq[1][64:128, ns],
                            in0=tmp1[64:128, :],
                            in1=tmp2[64:128, :],
                        )
                    elif mc == 1:
                        nc.vector.tensor_add(out=rot1[:, ns], in0=tmp1, in1=tmp2)
                    else:
                        nc.vector.tensor_add(out=rot2[:, ns], in0=tmp1, in1=tmp2)
                        m_ps = psSw.tile(
                            [64, 512], F32, tag="mir", bufs=1, name=f"m{nch}"
                        )
                        nc.tensor.matmul(
                            m_ps, mir_sb, rot2[:, ns], start=True, stop=True
                        )
                        nc.scalar.copy(out=rot3f[0:64, ns], in_=m_ps)
                        nc.gpsimd.tensor_copy(out=zq[2][0:64, ns], in_=rot2[0:64, ns])

                pending = None
                for nch in range(NCH):
                    ns = slice(512 * nch, 512 * (nch + 1))
                    for mc in range(3):
                        qk_ps = psA.tile(
                            [128, 512], F32, tag="qk", bufs=3, name=f"qk{mc}_{nch}"
                        )
                        for ci in range(KT):
                            nc.tensor.matmul(
                                qk_ps,
                                wqk_sb[:, ci, 128 * mc : 128 * (mc + 1)],
                                xt_sb[:, ci, ns],
                                start=(ci == 0),
                                stop=(ci == KT - 1),
                            )
                        qk_sb = evacp.tile(
                            [128, 512], BF16, tag="qkev", name=f"qkev{mc}_{nch}"
                        )
                        nc.scalar.copy(out=qk_sb, in_=qk_ps)
                        if pending is not None:
                            rope_tail(*pending)
                        pending = (mc, nch, qk_ps, qk_sb)
                    # V tiles for this slab overlap the rope tails on ACT/DVE
                    for ti in range(4 * nch, 4 * nch + 4):
                        v_ps = psV.tile([128, VW], F32, tag="vps", name=f"vps{ti}")
                        for ci in range(KT):
                            nc.tensor.matmul(
                                v_ps,
                                xt_sb[:, ci, 128 * ti : 128 * (ti + 1)],
                                wv_sb[:, ci, :],
                                start=(ci == 0),
                                stop=False,
                            )
                        nc.tensor.matmul(
                            v_ps,
                            ones_sb[0:1, 0:128],
                            bv_sb,
                            start=False,
                            stop=True,
                        )
                        nc.scalar.copy(out=v_sb[:, ti, :], in_=v_ps)
                rope_tail(*pending)

            # ============ Phase B: attention + interleaved projection ============
            # q-chunk-major: for each 512-wide q chunk, all three heads run
            # scores->exp, then the PV accumulation of iteration N runs after
            # iteration N+1's score matmuls (so the PE never stalls on the
            # Exp/affine_select of the last diagonal tile), the normalization
            # tail of N is likewise flushed one iteration late, and the
            # output projection of a finished q chunk follows its last
            # normalization flush.
            qsl = [zq[0], zq[1], zq[2]]
            ksl = [rot1, rot1, rot3f]
            # norm_tail output slot per head: (tile, row offset)
            ydst = [(yt01, 0), (yt01, 64), (yt2, 0)]
            with (
                tc.tile_pool(name="psS", bufs=2, space="PSUM") as psS,
                tc.tile_pool(name="psY", bufs=2, space="PSUM") as psY,
                tc.tile_pool(name="psP", bufs=2, space="PSUM") as psP,
                tc.tile_pool(name="pt", bufs=18) as ptp,
                tc.tile_pool(name="smax", bufs=3) as smaxp,
                tc.tile_pool(name="pout", bufs=4) as poutp,
                tc.tile_pool(name="rdram", bufs=3, space="DRAM") as rdp,
            ):

                def norm_head(hp, qc, y_ps, fast=False):
                    # r = exp(-ln(denom+sink)) on ACT: Ln/Exp/Copy share one
                    # activation table; the reciprocal stays off the DVE and
                    # off the PE critical path. The q-column broadcast runs
                    # as a DRAM round-trip (stride-0 partition read) on the
                    # idle gpsimd DMA queue — no PE matmul, no PSUM bank.
                    # fast=True (drain tail): rank-1 PE matmul into the
                    # unused partitions 64..127 instead — shorter latency.
                    lnr = smaxp.tile([1, 512], F32, tag="lnr", name=f"ln{hp}_{qc}")
                    nc.scalar.activation(
                        out=lnr,
                        in_=y_ps[64:65, :],
                        func=ACTF.Ln,
                        bias=sink_sb[0:1, 65 * hp + 64 : 65 * hp + 65],
                    )
                    r_sb = smaxp.tile([1, 512], BF16, tag="r", name=f"r{hp}_{qc}")
                    nc.scalar.activation(out=r_sb, in_=lnr, func=ACTF.Exp, scale=-1.0)
                    yu = smaxp.tile([64, 512], BF16, tag="yu", name=f"yu{hp}_{qc}")
                    nc.vector.tensor_copy(out=yu, in_=y_ps[0:64, :])
                    if fast:
                        nc.tensor.matmul(
                            y_ps[64:128, :],
                            ones_sb[0:1, 0:64],
                            r_sb,
                            start=True,
                            stop=True,
                        )
                        return (yu, y_ps[64:128, :])
                    scr = rdp.tile([1, 512], BF16, tag="scr", name=f"scr{hp}_{qc}")
                    nc.gpsimd.dma_start(out=scr, in_=r_sb)
                    rb_sb = smaxp.tile([64, 512], BF16, tag="rb", name=f"rb{hp}_{qc}")
                    nc.gpsimd.dma_start(
                        out=rb_sb,
                        in_=bass.AP(
                            tensor=scr.tensor, offset=scr.offset, ap=[[0, 64], [1, 512]]
                        ),
                    )
                    return (yu, rb_sb)

                def norm_finish(hp, qc, yu_rb):
                    yu, rb = yu_rb
                    dst, ro = ydst[hp]
                    nc.vector.tensor_mul(
                        out=dst[ro : ro + 64, 512 * qc : 512 * (qc + 1)],
                        in0=yu,
                        in1=rb,
                    )

                def emit_proj(qc):
                    for ti in range(4 * qc, 4 * qc + 4):
                        for nn in range(2):
                            p_ps = psP.tile(
                                [128, 384], F32, tag="p", name=f"p{ti}_{nn}"
                            )
                            nc.tensor.matmul(
                                p_ps,
                                yt01[:, 128 * ti : 128 * (ti + 1)],
                                wp_sb[:, 0, 384 * nn : 384 * (nn + 1)],
                                start=True,
                                stop=False,
                            )
                            nc.tensor.matmul(
                                p_ps,
                                yt2[:, 128 * ti : 128 * (ti + 1)],
                                wp_sb[:, 1, 384 * nn : 384 * (nn + 1)],
                                start=False,
                                stop=True,
                            )
                            p_sb = poutp.tile(
                                [128, 384], BF16, tag="psb", name=f"psb{ti}_{nn}"
                            )
                            if (ti + nn) % 2 == 0:
                                nc.vector.tensor_copy(out=p_sb, in_=p_ps)
                            else:
                                nc.scalar.copy(out=p_sb, in_=p_ps)
                            nc.sync.dma_start(
                                out=yp_d[
                                    128 * ti : 128 * (ti + 1),
                                    384 * nn : 384 * (nn + 1),
                                ],
                                in_=p_sb,
                            )

                def score_units(qc, hp, pts):
                    """Yield per-pair emitters: each emits 2 score matmuls +
                    1 Exp (+ affine_selects for diagonal pairs) and appends
                    to pts. Interleaving these with the previous iteration's
                    PV matmuls keeps ACT fed during the PV chain."""
                    qt = qsl[hp]
                    kt_ = ksl[hp]
                    nki = 4 * qc + 4
                    nfull = 4 * qc  # k-tiles with no causal masking

                    def full_pair(kp):
                        st2 = psS.tile(
                            [128, 1024],
                            F32,
                            tag="st2",
                            bufs=2,
                            name=f"st2_{hp}_{qc}_{kp}",
                        )
                        pt2 = ptp.tile(
                            [128, 1024],
                            BF16,
                            tag="pt",
                            name=f"pt_{hp}_{qc}_{kp}",
                            bufs=14,
                        )
                        for j in range(2):
                            ki = kp + j
                            nc.tensor.matmul(
                                st2[:, 512 * j : 512 * (j + 1)],
                                kt_[:, 128 * ki : 128 * (ki + 1)],
                                qt[:, 512 * qc : 512 * (qc + 1)],
                                start=True,
                                stop=True,
                            )
                        nc.scalar.activation(out=pt2, in_=st2, func=ACTF.Exp)
                        pts.append((pt2[:, 0:512], 0))
                        pts.append((pt2[:, 512:1024], 0))

                    def masked_pair(kp):
                        lefts = [
                            max(0, 128 * (kp + j) - 512 * qc) for j in range(2)
                        ]
                        widths = [512 - lf for lf in lefts]
                        tot = widths[0] + widths[1]
                        st_ps = psS.tile(
                            [128, 1024],
                            F32,
                            tag="st2",
                            bufs=2,
                            name=f"st{hp}_{qc}_{kp}",
                        )
                        ptm = ptp.tile(
                            [128, 1024],
                            BF16,
                            tag="ptm",
                            name=f"ptm_{hp}_{qc}_{kp}",
                            bufs=4,
                        )
                        off = 0
                        sub = []
                        for j in range(2):
                            ki = kp + j
                            nc.tensor.matmul(
                                st_ps[:, off : off + widths[j]],
                                kt_[:, 128 * ki : 128 * (ki + 1)],
                                qt[:, 512 * qc + lefts[j] : 512 * (qc + 1)],
                                start=True,
                                stop=True,
                            )
                            sub.append((off, widths[j], lefts[j]))
                            off += widths[j]
                        nc.scalar.activation(
                            out=ptm[:, :tot], in_=st_ps[:, :tot], func=ACTF.Exp
                        )
                        for j in range(2):
                            off_j, w_j, lf_j = sub[j]
                            nc.gpsimd.affine_select(
                                out=ptm[:, off_j : off_j + 128],
                                in_=ptm[:, off_j : off_j + 128],
                                pattern=[[1, 128]],
                                base=0,
                                channel_multiplier=-1,
                                compare_op=ALU.is_ge,
                                fill=0.0,
                            )
                            pts.append((ptm[:, off_j : off_j + w_j], lf_j))

                    for kp in range(0, nfull, 2):
                        yield lambda kp=kp: full_pair(kp)
                    for kp in range(nfull, nki, 2):
                        yield lambda kp=kp: masked_pair(kp)

                def pv_units(qc, hp, pts, y_ps):
                    nki = 4 * qc + 4

                    def one(ki):
                        rhs_ap, left = pts[ki]
                        nc.tensor.matmul(
                            y_ps[0:65, left:],
                            v_sb[:, ki, 65 * hp : 65 * hp + 65],
                            rhs_ap,
                            start=(ki == 0),
                            stop=(ki == nki - 1),
                            skip_group_check=True,
                        )

                    for ki in range(nki):
                        yield lambda ki=ki: one(ki)

                iters = [(qc, hp) for qc in range(NCH) for hp in range(3)]
                prev = None  # (qc, hp, pts) awaiting PV
                pending_fin = None

                def tail_stage(pqc, php, y_ps, fast=False):
                    # norm head for the PV that just finished; flush the
                    # finish + projection of the one before it
                    nonlocal pending_fin
                    yu_rb = norm_head(php, pqc, y_ps, fast=fast)
                    if pending_fin is not None:
                        norm_finish(*pending_fin)
                        if pending_fin[0] == 2:
                            emit_proj(pending_fin[1])
                    pending_fin = (php, pqc, yu_rb)

                last_iter = iters[-1]
                for qc, hp in iters:
                    pts = []
                    sus = list(score_units(qc, hp, pts))
                    if prev is not None:
                        pqc, php, ppts = prev
                        y_ps = psY.tile(
                            [128, 512], F32, tag="y", name=f"y{php}_{pqc}"
                        )
                        pvs = list(pv_units(pqc, php, ppts, y_ps))
                    else:
                        y_ps = None
                        pvs = []
                    # interleave: 1 score pair then ~2 PV matmuls so the
                    # Exp queue on ACT never runs dry during the PV chain
                    nsu, npv = len(sus), len(pvs)
                    pi = 0
                    for si, su in enumerate(sus):
                        su()
                        target = (si + 1) * npv // nsu if nsu else npv
                        while pi < target:
                            pvs[pi]()
                            pi += 1
                    while pi < npv:
                        pvs[pi]()
                        pi += 1
                    if prev is not None:
                        tail_stage(prev[0], prev[1], y_ps, fast=(qc, hp) == last_iter)
                    prev = (qc, hp, pts)
                # drain the pipeline
                pqc, php, ppts = prev
                y_ps = psY.tile([128, 512], F32, tag="y", name=f"y{php}_{pqc}")
                for pv in pv_units(pqc, php, ppts, y_ps):
                    pv()
                tail_stage(pqc, php, y_ps, fast=True)
                norm_finish(*pending_fin)
                emit_proj(NCH - 1)

    _split_waits(nc)
    return nc


_NC_CACHE = {}


def _get_nc():
    if "nc" not in _NC_CACHE:
        _NC_CACHE["nc"] = build_nc()
    return _NC_CACHE["nc"]


def _prep_core_inputs(inputs):
    """Host-side sharding: fold norm/scale/rezero into weights, build the
    per-core input maps."""
    bf16 = ml_dtypes.bfloat16
    x = np.asarray(inputs["x"], np.float32)
    ns_ = np.asarray(inputs["norm_scale"], np.float32)
    nb_ = np.asarray(inputs["norm_bias"], np.float32)
    Wa = np.asarray(inputs["W_attn"], np.float32)
    ba = np.asarray(inputs["b_attn"], np.float32)
    Wp = np.asarray(inputs["W_proj"], np.float32)
    sinks = np.asarray(inputs["sinks"], np.float32)
    rz = float(np.asarray(inputs["rezero"], np.float32).reshape(()))

    C = N_EMBD
    W = (ns_[:, None] * Wa).astype(np.float32)
    beff = (nb_.astype(np.float64) @ Wa.astype(np.float64) + ba).astype(np.float32)
    scale = 1.0 / np.sqrt(np.float32(HD))

    # RoPE tables, interleaved-row layout (64-row periodic)
    freqs = 1.0 / THETA ** (np.arange(0, HD, 2, dtype=np.float64) / HD)  # [32]
    tpos = np.arange(T, dtype=np.float64)
    ang = np.outer(tpos, freqs)  # [T, 32]
    cos_t = np.cos(ang).T  # [32, T]
    sin_t = np.sin(ang).T
    ct = np.empty((128, T), np.float64)
    st = np.empty((128, T), np.float64)
    for blk in range(2):
        r0 = 64 * blk
        ct[r0 + 0 : r0 + 64 : 2] = cos_t
        ct[r0 + 1 : r0 + 64 : 2] = cos_t
        st[r0 + 0 : r0 + 64 : 2] = -sin_t
        st[r0 + 1 : r0 + 64 : 2] = sin_t
    ct = ct.astype(bf16)
    st = st.astype(bf16)

    perm = np.zeros((128, 128), np.float32)
    for i in range(64):
        perm[2 * i, 2 * i + 1] = 1.0
        perm[2 * i + 1, 2 * i] = 1.0
    mir = np.zeros((128, 64), np.float32)
    for i in range(64):
        mir[64 + i, i] = 1.0
    ones = np.ones((1, 512), np.float32)
    swap_idx = np.arange(128)
    swap_idx = swap_idx + 1 - 2 * (swap_idx % 2)  # pairwise swap

    in_maps = []
    for d in range(NCORES):
        b = d // 4
        g = d % 4
        heads = [3 * g + j for j in range(HL)]

        wqk = np.empty((C, 384), np.float32)
        bqk = np.empty((128, 3), np.float32)
        # c0 = [q_h0 | q_h1], c1 = [k_h0 | k_h1], c2 = [q_h2 | k_h2]
        h0, h1, h2 = heads
        wqk[:, 0:64] = W[:, 64 * h0 : 64 * h0 + 64] * scale
        wqk[:, 64:128] = W[:, 64 * h1 : 64 * h1 + 64] * scale
        wqk[:, 128:192] = W[:, C + 64 * h0 : C + 64 * h0 + 64]
        wqk[:, 192:256] = W[:, C + 64 * h1 : C + 64 * h1 + 64]
        wqk[:, 256:320] = W[:, 64 * h2 : 64 * h2 + 64] * scale
        wqk[:, 320:384] = W[:, C + 64 * h2 : C + 64 * h2 + 64]
        bqk[0:64, 0] = beff[64 * h0 : 64 * h0 + 64] * scale
        bqk[64:128, 0] = beff[64 * h1 : 64 * h1 + 64] * scale
        bqk[0:64, 1] = beff[C + 64 * h0 : C + 64 * h0 + 64]
        bqk[64:128, 1] = beff[C + 64 * h1 : C + 64 * h1 + 64]
        bqk[0:64, 2] = beff[64 * h2 : 64 * h2 + 64] * scale
        bqk[64:128, 2] = beff[C + 64 * h2 : C + 64 * h2 + 64]
        bqksw = bqk[swap_idx, :].copy()

        wv = np.zeros((C, VW), np.float32)
        bv = np.zeros((1, VW), np.float32)
        for j, h in enumerate(heads):
            wv[:, 65 * j : 65 * j + 64] = W[:, 2 * C + 64 * h : 2 * C + 64 * h + 64]
            bv[0, 65 * j : 65 * j + 64] = beff[2 * C + 64 * h : 2 * C + 64 * h + 64]
            bv[0, 65 * j + 64] = 1.0

        # projection weights: heads 0,1 packed on partitions; head 2 + pad
        wp = np.zeros((128, 2, C), np.float32)
        wp[0:64, 0, :] = Wp[64 * h0 : 64 * h0 + 64, :] * rz
        wp[64:128, 0, :] = Wp[64 * h1 : 64 * h1 + 64, :] * rz
        wp[0:64, 1, :] = Wp[64 * h2 : 64 * h2 + 64, :] * rz

        sinkrow = np.zeros((1, VW), np.float32)
        for j, h in enumerate(heads):
            sinkrow[0, 65 * j + 64] = np.exp(np.float64(sinks[h]))

        in_maps.append(
            {
                "xt": np.ascontiguousarray(x[b].T).astype(bf16),
                "wqk": wqk.astype(bf16),
                "bqk": bqk,
                "bqksw": bqksw,
                "wv": wv.astype(bf16),
                "bv": bv.astype(bf16),
                "wp": wp.astype(bf16),
                "ct": ct,
                "st": st,
                "perm": perm.astype(bf16),
                "mir": mir.astype(bf16),
                "sinkrow": sinkrow,
                "ones": ones.astype(bf16),
            }
        )

    bias_out = (np.asarray(inputs["b_proj"], np.float32) * rz).astype(np.float32)
    return in_maps, bias_out


def kernel(**inputs):
    global LAST_RESULTS
    nc = _get_nc()
    in_maps, bias_out = _prep_core_inputs(inputs)
    res = None
    last_exc = None
    for attempt in range(3):
        try:
            res = run_bass_kernel_spmd(nc, in_maps, core_ids=list(range(NCORES)))
            break
        except Exception as e:  # transient NRT_EXEC_UNIT_UNRECOVERABLE etc.
            last_exc = e
            import time as _time

            _time.sleep(2.0)
    if res is None:
        raise last_exc
    LAST_RESULTS = res
    y = np.zeros((B, T, N_EMBD), np.float32)
    for d in range(NCORES):
        y[d // 4] += np.asarray(res.results[d]["yp"], dtype=np.float32)
    y += bias_out[None, None, :]
    return y


# revision 26
# speedup vs baseline: 1.0647x; 1.0502x over previous
"""Causal self-attention with RoPE + attention sinks on 8 Trainium2 NeuronCores.

Sharding: core d handles batch b = d//4 and heads 3*(d%4) .. 3*(d%4)+2
(data parallel on B x tensor parallel on the 12 heads). Each core computes a
partial [T, C] projection output (c_proj contraction over its 3 heads); the
host sums the 4 head-group partials per batch and adds b_proj*rezero.

Device kernel layout choices (bf16 datapath):
 - All matmul operands are bf16 (1 PE cycle/row, same as f32r, but half the
   DMA/SBUF footprint and 2x DVE); accumulation stays fp32 in PSUM.
 - Phase A is slab-pipelined: xt arrives in 512-column slabs and the QKV
   matmuls for a slab start as soon as it lands, so the PE starts ~4us in
   instead of waiting for the full activation load.
 - qT/kT are computed directly in [d, t] (transposed) layout so scores^T
   tiles [k_tile, q] come out PE-native; softmax exp runs on ACT PSUM->SBUF,
   causal masking via gpsimd affine_select, and the PV matmul needs no
   transposes (lhsT = V tile, rhs = P^T tile).
 - The softmax denominator is a free 65th column of V (ones); the reciprocal
   runs as Ln/Exp on ACT; the q-column broadcast of the reciprocal runs as a
   DRAM round-trip DMA (stride-0 partition read) on the gpsimd queue, with a
   rank-1 PE matmul fallback for the drain tail.
 - RoPE: pair-swap via a permutation-matrix matmul on the PE; the two
   multiply-adds run as fused scalar_tensor_tensor DVE ops which also fold in
   the QKV bias (bias and pair-swapped bias as per-partition scalars).
 - Phase B runs q-chunk-major with software pipelining: the PV chain of
   iteration N is interleaved 2:1 between iteration N+1's score pairs so the
   ACT Exp queue never runs dry; the projection for a q-chunk is emitted as
   soon as its three heads are normalized, overlapping projection + output
   DMA with the remaining attention work.
 - yT is packed two heads per 128 partitions, halving projection matmuls.
 - Norm affine, 1/sqrt(hd), and rezero are folded into weights on the host.
"""

import numpy as np
import ml_dtypes

import concourse.bass as bass
import concourse.mybir as mybir
import concourse.tile as tile
from concourse.vector_clock import ScopedClock
from concourse.bass_utils import run_bass_kernel_spmd

F32 = mybir.dt.float32
BF16 = mybir.dt.bfloat16
ALU = mybir.AluOpType
ACTF = mybir.ActivationFunctionType

N_HEAD = 12
N_EMBD = 768
T = 2048
B = 2
HD = 64
HL = 3  # heads per core
NCORES = 8
THETA = 10000.0

KT = 6  # contraction tiles over C=768
NCH = 4  # 512-wide column chunks over T
TTILES = 16  # 128-row tiles over T
VW = 195  # v columns: 3 heads x (64 dims + ones col)

LAST_RESULTS = None  # BassKernelResults of the most recent run (for test.py)


class _TC(tile.TileContext):
    """TileContext whose tail drain splits its sem waits across multiple
    drain instructions; this walrus build rejects >1 wait on an SP Drain."""

    def _drain_and_barrier(self, tick_clock, wait_clock):
        drain_inst = self.nc.sync.drain()
        wait_clock.add_sem_waits(
            drain_inst.ins, ScopedClock({None: tick_clock.global_clock})
        )
        si = drain_inst.ins.sync_info
        if si is not None and len(si.on_wait) > 1:
            waits = list(si.on_wait)
            si.on_wait = waits[:1]
            for w in waits[1:]:
                d2 = self.nc.sync.drain()
                d2.ins.sync_info = mybir.SyncInfo(on_wait=[w], on_update=[])
        self.nc.all_engine_barrier()
        assert self.sems is not None
        popped = self.nc._tile_sem_poison_stack.pop()
        assert popped is self._sem_poison
        self.nc.clear_and_free_semaphores(list(self.sems.allocated().values()))
        self.nc.all_engine_barrier()


def _split_waits(nc, max_waits=1):
    """This walrus build accepts at most one sem-wait per TPB instruction.
    Hoist excess waits of any instruction onto NoOps inserted just before it
    on the same engine (in-order execution keeps semantics identical)."""
    for blk in nc.m.functions[0].blocks:
        new_insts = []
        for inst in blk.instructions:
            si = inst.sync_info
            if si is not None and len(si.on_wait) > max_waits:
                waits = list(si.on_wait)
                extra, keep = waits[:-max_waits], waits[-max_waits:]
                for i in range(0, len(extra), max_waits):
                    nop = mybir.InstNoOp(
                        name=nc.get_next_instruction_name(),
                        engine=inst.engine,
                        ins=[],
                        outs=[],
                        sync_info=mybir.SyncInfo(
                            on_wait=extra[i : i + max_waits], on_update=[]
                        ),
                    )
                    nc.register_instruction(nop)
                    new_insts.append(nop)
                si.on_wait = keep
            new_insts.append(inst)
        blk.instructions[:] = new_insts
    return nc


def build_nc():
    nc = bass.Bass()

    xt_d = nc.dram_tensor("xt", [N_EMBD, T], BF16, kind="ExternalInput")
    wqk_d = nc.dram_tensor("wqk", [N_EMBD, 384], BF16, kind="ExternalInput")
    bqk_d = nc.dram_tensor("bqk", [128, 3], F32, kind="ExternalInput")
    bqksw_d = nc.dram_tensor("bqksw", [128, 3], F32, kind="ExternalInput")
    wv_d = nc.dram_tensor("wv", [N_EMBD, VW], BF16, kind="ExternalInput")
    bv_d = nc.dram_tensor("bv", [1, VW], BF16, kind="ExternalInput")
    wp_d = nc.dram_tensor("wp", [128, 2, N_EMBD], BF16, kind="ExternalInput")
    ct_d = nc.dram_tensor("ct", [128, T], BF16, kind="ExternalInput")
    st_d = nc.dram_tensor("st", [128, T], BF16, kind="ExternalInput")
    perm_d = nc.dram_tensor("perm", [128, 128], BF16, kind="ExternalInput")
    mir_d = nc.dram_tensor("mir", [128, 64], BF16, kind="ExternalInput")
    sink_d = nc.dram_tensor("sinkrow", [1, VW], F32, kind="ExternalInput")
    ones_d = nc.dram_tensor("ones", [1, 512], BF16, kind="ExternalInput")
    yp_d = nc.dram_tensor("yp", [T, N_EMBD], BF16, kind="ExternalOutput")

    with _TC(nc) as tc:
        with (
            tc.tile_pool(name="consts", bufs=1) as consts,
            tc.tile_pool(name="rot", bufs=1) as rotp,
            tc.tile_pool(name="vsb", bufs=1) as vp,
            tc.tile_pool(name="ytp", bufs=1) as ytp,
        ):
            # ---- persistent tiles ----
            ones_sb = consts.tile([1, 512], BF16, tag="ones")
            sink_sb = consts.tile([1, VW], F32, tag="sink")
            wp_sb = consts.tile([128, 2, N_EMBD], BF16, tag="wp")

            # rot1: rotated [k_h0|k_h1]; rot2: rotated [q_h2|k_h2];
            # rot3f: [k_h2 relocated to 0..63 | zeros].
            # zq0/1/2: per-head rotated q zero-padded to 128 partitions so the
            # scores matmuls contract over K=128 (the PE clock governor
            # ignores half-array K=64 matmuls and throttles down).
            rot1 = rotp.tile([128, T], BF16, tag="rot1")
            rot2 = rotp.tile([128, T], BF16, tag="rot2")
            rot3f = rotp.tile([128, T], BF16, tag="rot3f")
            zq = [
                rotp.tile([128, T], BF16, tag=f"zq{i}", name=f"zq{i}")
                for i in range(3)
            ]
            # v tiles [t-tile 128, 3 heads x (64 dims + ones col)]
            v_sb = vp.tile([128, TTILES, VW], BF16, tag="v")
            # yt01: heads 0,1 packed on partitions; yt2: head 2 + zero pad
            yt01 = ytp.tile([128, T], BF16, tag="yt01")
            yt2 = ytp.tile([128, T], BF16, tag="yt2")

            # ================= Phase A: QKV + RoPE + V =================
            # Slab-pipelined: xt lands in 512-column slabs; each slab's QKV
            # matmuls + rope tails + V tiles run while the next slab loads.
            # The rope tail of chunk N is emitted after chunk N+1's QKV
            # matmuls so the PE never stalls on the ACT evacuation of N.
            with (
                tc.tile_pool(name="phA", bufs=1) as phA,
                tc.tile_pool(name="psA", bufs=3, space="PSUM") as psA,
                tc.tile_pool(name="psSw", bufs=3, space="PSUM") as psSw,
                tc.tile_pool(name="psV", bufs=2, space="PSUM") as psV,
                tc.tile_pool(name="evac", bufs=3) as evacp,
                tc.tile_pool(name="tmps", bufs=3) as tmpp,
            ):
                # critical-path loads on the SP queue: wqk, then the xt
                # slabs. Everything else issues from the scalar / gpsimd
                # DGE queues so slab N+1's descriptors are never stuck
                # behind a pile of small-constant issues.
                # Big critical loads split in halves so the transfers spread
                # across more DMA engines (one dma_start saturates only a
                # few rings).
                wqk_sb = phA.tile([128, KT, 384], BF16, tag="wqk")
                wqk_r = wqk_d.rearrange("(ci p) m -> p ci m", p=128)
                nc.sync.dma_start(out=wqk_sb[:, 0:2, :], in_=wqk_r[:, 0:2, :])
                nc.scalar.dma_start(out=wqk_sb[:, 2:4, :], in_=wqk_r[:, 2:4, :])
                nc.sync.dma_start(out=wqk_sb[:, 4:6, :], in_=wqk_r[:, 4:6, :])
                xt_sb = phA.tile([128, KT, T], BF16, tag="xt")
                xt_r = xt_d.rearrange("(ci p) m -> p ci m", p=128)
                ct_sb = phA.tile([128, T], BF16, tag="ct")
                st_sb = phA.tile([128, T], BF16, tag="st")

                for nch in range(NCH):
                    ns = slice(512 * nch, 512 * (nch + 1))
                    nc.sync.dma_start(out=xt_sb[:, 0:2, ns], in_=xt_r[:, 0:2, ns])
                    nc.scalar.dma_start(
                        out=xt_sb[:, 2:4, ns], in_=xt_r[:, 2:4, ns]
                    )
                    nc.sync.dma_start(out=xt_sb[:, 4:6, ns], in_=xt_r[:, 4:6, ns])
                # rope tables + small constants on the gpsimd queue,
                # interleaved with the zero-pad memsets so their transfers
                # don't pile onto the critical xt slabs all at once
                bqk_sb = phA.tile([128, 3], F32, tag="bqk")
                nc.gpsimd.dma_start(out=bqk_sb, in_=bqk_d[:, :])
                bqksw_sb = phA.tile([128, 3], F32, tag="bqksw")
                nc.gpsimd.dma_start(out=bqksw_sb, in_=bqksw_d[:, :])
                perm_sb = phA.tile([128, 128], BF16, tag="perm")
                nc.gpsimd.dma_start(out=perm_sb, in_=perm_d[:, :])
                mir_sb = phA.tile([128, 64], BF16, tag="mir")
                nc.gpsimd.dma_start(out=mir_sb, in_=mir_d[:, :])
                nc.gpsimd.dma_start(out=ones_sb, in_=ones_d[:, :])
                nc.gpsimd.dma_start(out=sink_sb, in_=sink_d[:, :])
                nc.gpsimd.dma_start(out=ct_sb[:, 0:512], in_=ct_d[:, 0:512])
                nc.gpsimd.dma_start(out=st_sb[:, 0:512], in_=st_d[:, 0:512])
                nc.gpsimd.memset(zq[0][64:128, :], 0.0)
                nc.gpsimd.dma_start(out=ct_sb[:, 512:1024], in_=ct_d[:, 512:1024])
                nc.gpsimd.dma_start(out=st_sb[:, 512:1024], in_=st_d[:, 512:1024])
                nc.gpsimd.memset(zq[1][0:64, :], 0.0)
                nc.gpsimd.dma_start(out=ct_sb[:, 1024:2048], in_=ct_d[:, 1024:2048])
                nc.gpsimd.dma_start(out=st_sb[:, 1024:2048], in_=st_d[:, 1024:2048])
                nc.gpsimd.memset(zq[2][64:128, :], 0.0)
                nc.gpsimd.memset(rot3f[64:128, :], 0.0)
                nc.gpsimd.memset(yt2[64:128, :], 0.0)
                # weights for V / projection on the scalar queue
                wv_sb = phA.tile([128, KT, VW], BF16, tag="wv")
                nc.scalar.dma_start(
                    out=wv_sb, in_=wv_d.rearrange("(ci p) m -> p ci m", p=128)
                )
                bv_sb = phA.tile([1, VW], BF16, tag="bv")
                nc.scalar.dma_start(out=bv_sb, in_=bv_d[:, :])
                nc.scalar.dma_start(out=wp_sb, in_=wp_d[:, :, :])

                def rope_tail(mc, nch, qk_ps, qk_sb):
                    ns = slice(512 * nch, 512 * (nch + 1))
                    sw_ps = psSw.tile(
                        [128, 512], F32, tag="sw", bufs=2, name=f"sw{mc}_{nch}"
                    )
                    nc.tensor.matmul(sw_ps, perm_sb, qk_sb, start=True, stop=True)
                    sw_sb = tmpp.tile(
                        [128, 512], BF16, tag="swsb", name=f"swsb{mc}_{nch}"
                    )
                    nc.scalar.copy(out=sw_sb, in_=sw_ps)
                    tmp1 = tmpp.tile([128, 512], BF16, tag="t1", name=f"t1_{mc}_{nch}")
                    nc.vector.scalar_tensor_tensor(
                        out=tmp1,
                        in0=qk_sb,
                        scalar=bqk_sb[:, mc : mc + 1],
                        in1=ct_sb[:, ns],
                        op0=ALU.add,
                        op1=ALU.mult,
                    )
                    tmp2 = tmpp.tile([128, 512], BF16, tag="t2", name=f"t2_{mc}_{nch}")
                    nc.vector.scalar_tensor_tensor(
                        out=tmp2,
                        in0=sw_sb,
                        scalar=bqksw_sb[:, mc : mc + 1],
                        in1=st_sb[:, ns],
                        op0=ALU.add,
                        op1=ALU.mult,
                    )
                    if mc == 0:
                        nc.vector.tensor_add(
                            out=zq[0][0:64, ns], in0=tmp1[0:64, :], in1=tmp2[0:64, :]
                        )
                        nc.vector.tensor_add(
                            out=zq[1][64:128, ns],
                            in0=tmp1[64:128, :],
                            in1=tmp2[64:128, :],
                        )
                    elif mc == 1:
                        nc.vector.tensor_add(out=rot1[:, ns], in0=tmp1, in1=tmp2)
                    else:
                        nc.vector.tensor_add(out=rot2[:, ns], in0=tmp1, in1=tmp2)
                        m_ps = psSw.tile(
                            [64, 512], F32, tag="mir", bufs=1, name=f"m{nch}"
                        )
                        nc.tensor.matmul(
                            m_ps, mir_sb, rot2[:, ns], start=True, stop=True
                        )
                        nc.vector.tensor_copy(out=rot3f[0:64, ns], in_=m_ps)
                        nc.gpsimd.tensor_copy(out=zq[2][0:64, ns], in_=rot2[0:64, ns])

                pending = None
                for nch in range(NCH):
                    ns = slice(512 * nch, 512 * (nch + 1))
                    for mc in range(3):
                        qk_ps = psA.tile(
                            [128, 512], F32, tag="qk", bufs=3, name=f"qk{mc}_{nch}"
                        )
                        for ci in range(KT):
                            nc.tensor.matmul(
                                qk_ps,
                                wqk_sb[:, ci, 128 * mc : 128 * (mc + 1)],
                                xt_sb[:, ci, ns],
                                start=(ci == 0),
                                stop=(ci == KT - 1),
                            )
                        qk_sb = evacp.tile(
                            [128, 512], BF16, tag="qkev", name=f"qkev{mc}_{nch}"
                        )
                        nc.scalar.copy(out=qk_sb, in_=qk_ps)
                        if pending is not None:
                            rope_tail(*pending)
                        pending = (mc, nch, qk_ps, qk_sb)
                    # V tiles for this slab overlap the rope tails on ACT/DVE
                    for ti in range(4 * nch, 4 * nch + 4):
                        v_ps = psV.tile([128, VW], F32, tag="vps", name=f"vps{ti}")
                        for ci in range(KT):
                            nc.tensor.matmul(
                                v_ps,
                                xt_sb[:, ci, 128 * ti : 128 * (ti + 1)],
                                wv_sb[:, ci, :],
                                start=(ci == 0),
                                stop=False,
                            )
                        nc.tensor.matmul(
                            v_ps,
                            ones_sb[0:1, 0:128],
                            bv_sb,
                            start=False,
                            stop=True,
                        )
                        nc.vector.tensor_copy(out=v_sb[:, ti, :], in_=v_ps)
                rope_tail(*pending)

            # ============ Phase B: attention + interleaved projection ============
            # q-chunk-major: for each 512-wide q chunk, all three heads run
            # scores->exp, then the PV accumulation of iteration N runs after
            # iteration N+1's score matmuls (so the PE never stalls on the
            # Exp/affine_select of the last diagonal tile), the normalization
            # tail of N is likewise flushed one iteration late, and the
            # output projection of a finished q chunk follows its last
            # normalization flush.
            qsl = [zq[0], zq[1], zq[2]]
            ksl = [rot1, rot1, rot3f]
            # norm_tail output slot per head: (tile, row offset)
            ydst = [(yt01, 0), (yt01, 64), (yt2, 0)]
            with (
                tc.tile_pool(name="psS", bufs=2, space="PSUM") as psS,
                tc.tile_pool(name="psY", bufs=2, space="PSUM") as psY,
                tc.tile_pool(name="psP", bufs=2, space="PSUM") as psP,
                tc.tile_pool(name="pt", bufs=18) as ptp,
                tc.tile_pool(name="smax", bufs=3) as smaxp,
                tc.tile_pool(name="pout", bufs=4) as poutp,
                tc.tile_pool(name="rdram", bufs=3, space="DRAM") as rdp,
            ):

                def norm_head(hp, qc, y_ps, fast=False):
                    # r = exp(-ln(denom+sink)) on ACT: Ln/Exp/Copy share one
                    # activation table; the reciprocal stays off the DVE and
                    # off the PE critical path. The q-column broadcast runs
                    # as a DRAM round-trip (stride-0 partition read) on the
                    # idle gpsimd DMA queue — no PE matmul, no PSUM bank.
                    # fast=True (drain tail): rank-1 PE matmul into the
                    # unused partitions 64..127 instead — shorter latency.
                    lnr = smaxp.tile([1, 512], F32, tag="lnr", name=f"ln{hp}_{qc}")
                    nc.scalar.activation(
                        out=lnr,
                        in_=y_ps[64:65, :],
                        func=ACTF.Ln,
                        bias=sink_sb[0:1, 65 * hp + 64 : 65 * hp + 65],
                    )
                    r_sb = smaxp.tile([1, 512], BF16, tag="r", name=f"r{hp}_{qc}")
                    nc.scalar.activation(out=r_sb, in_=lnr, func=ACTF.Exp, scale=-1.0)
                    yu = smaxp.tile([64, 512], BF16, tag="yu", name=f"yu{hp}_{qc}")
                    nc.vector.tensor_copy(out=yu, in_=y_ps[0:64, :])
                    if fast:
                        nc.tensor.matmul(
                            y_ps[64:128, :],
                            ones_sb[0:1, 0:64],
                            r_sb,
                            start=True,
                            stop=True,
                        )
                        return (yu, y_ps[64:128, :])
                    scr = rdp.tile([1, 512], BF16, tag="scr", name=f"scr{hp}_{qc}")
                    nc.gpsimd.dma_start(out=scr, in_=r_sb)
                    rb_sb = smaxp.tile([64, 512], BF16, tag="rb", name=f"rb{hp}_{qc}")
                    nc.gpsimd.dma_start(
                        out=rb_sb,
                        in_=bass.AP(
                            tensor=scr.tensor, offset=scr.offset, ap=[[0, 64], [1, 512]]
                        ),
                    )
                    return (yu, rb_sb)

                def norm_finish(hp, qc, yu_rb):
                    yu, rb = yu_rb
                    dst, ro = ydst[hp]
                    nc.vector.tensor_mul(
                        out=dst[ro : ro + 64, 512 * qc : 512 * (qc + 1)],
                        in0=yu,
                        in1=rb,
                    )

                def emit_proj(qc):
                    tail_chunk = qc == NCH - 1
                    for ti in range(4 * qc, 4 * qc + 4):
                        for nn in range(2):
                            p_ps = psP.tile(
                                [128, 384], F32, tag="p", name=f"p{ti}_{nn}"
                            )
                            nc.tensor.matmul(
                                p_ps,
                                yt01[:, 128 * ti : 128 * (ti + 1)],
                                wp_sb[:, 0, 384 * nn : 384 * (nn + 1)],
                                start=True,
                                stop=False,
                            )
                            nc.tensor.matmul(
                                p_ps,
                                yt2[:, 128 * ti : 128 * (ti + 1)],
                                wp_sb[:, 1, 384 * nn : 384 * (nn + 1)],
                                start=False,
                                stop=True,
                            )
                            p_sb = poutp.tile(
                                [128, 384], BF16, tag="psb", name=f"psb{ti}_{nn}"
                            )
                            if tail_chunk and (ti + nn) % 2 == 1:
                                nc.scalar.copy(out=p_sb, in_=p_ps)
                            else:
                                nc.vector.tensor_copy(out=p_sb, in_=p_ps)
                            nc.sync.dma_start(
                                out=yp_d[
                                    128 * ti : 128 * (ti + 1),
                                    384 * nn : 384 * (nn + 1),
                                ],
                                in_=p_sb,
                            )

                def score_units(qc, hp, pts):
                    """Yield per-pair emitters: each emits 2 score matmuls +
                    1 Exp (+ affine_selects for diagonal pairs) and appends
                    to pts. Interleaving these with the previous iteration's
                    PV matmuls keeps ACT fed during the PV chain."""
                    qt = qsl[hp]
                    kt_ = ksl[hp]
                    nki = 4 * qc + 4
                    nfull = 4 * qc  # k-tiles with no causal masking

                    def full_pair(kp):
                        st2 = psS.tile(
                            [128, 1024],
                            F32,
                            tag="st2",
                            bufs=2,
                            name=f"st2_{hp}_{qc}_{kp}",
                        )
                        pt2 = ptp.tile(
                            [128, 1024],
                            BF16,
                            tag="pt",
                            name=f"pt_{hp}_{qc}_{kp}",
                            bufs=14,
                        )
                        for j in range(2):
                            ki = kp + j
                            nc.tensor.matmul(
                                st2[:, 512 * j : 512 * (j + 1)],
                                kt_[:, 128 * ki : 128 * (ki + 1)],
                                qt[:, 512 * qc : 512 * (qc + 1)],
                                start=True,
                                stop=True,
                            )
                        nc.scalar.activation(out=pt2, in_=st2, func=ACTF.Exp)
                        pts.append((pt2[:, 0:512], 0))
                        pts.append((pt2[:, 512:1024], 0))

                    def masked_pair(kp):
                        lefts = [
                            max(0, 128 * (kp + j) - 512 * qc) for j in range(2)
                        ]
                        widths = [512 - lf for lf in lefts]
                        tot = widths[0] + widths[1]
                        st_ps = psS.tile(
                            [128, 1024],
                            F32,
                            tag="st2",
                            bufs=2,
                            name=f"st{hp}_{qc}_{kp}",
                        )
                        ptm = ptp.tile(
                            [128, 1024],
                            BF16,
                            tag="ptm",
                            name=f"ptm_{hp}_{qc}_{kp}",
                            bufs=4,
                        )
                        off = 0
                        sub = []
                        for j in range(2):
                            ki = kp + j
                            nc.tensor.matmul(
                                st_ps[:, off : off + widths[j]],
                                kt_[:, 128 * ki : 128 * (ki + 1)],
                                qt[:, 512 * qc + lefts[j] : 512 * (qc + 1)],
                                start=True,
                                stop=True,
                            )
                            sub.append((off, widths[j], lefts[j]))
                            off += widths[j]
                        nc.scalar.activation(
                            out=ptm[:, :tot], in_=st_ps[:, :tot], func=ACTF.Exp
                        )
                        for j in range(2):
                            off_j, w_j, lf_j = sub[j]
                            nc.gpsimd.affine_select(
                                out=ptm[:, off_j : off_j + 128],
                                in_=ptm[:, off_j : off_j + 128],
                                pattern=[[1, 128]],
                                base=0,
                                channel_multiplier=-1,
                                compare_op=ALU.is_ge,
                                fill=0.0,
                            )
                            pts.append((ptm[:, off_j : off_j + w_j], lf_j))

                    for kp in range(0, nfull, 2):
                        yield lambda kp=kp: full_pair(kp)
                    for kp in range(nfull, nki, 2):
                        yield lambda kp=kp: masked_pair(kp)

                def pv_units(qc, hp, pts, y_ps):
                    nki = 4 * qc + 4

                    def one(ki):
                        rhs_ap, left = pts[ki]
                        nc.tensor.matmul(
                            y_ps[0:65, left:],
                            v_sb[:, ki, 65 * hp : 65 * hp + 65],
                            rhs_ap,
                            start=(ki == 0),
                            stop=(ki == nki - 1),
                            skip_group_check=True,
                        )

                    for ki in range(nki):
                        yield lambda ki=ki: one(ki)

                iters = [(qc, hp) for qc in range(NCH) for hp in range(3)]
                prev = None  # (qc, hp, pts) awaiting PV
                pending_fin = None

                def tail_stage(pqc, php, y_ps, fast=False):
                    # norm head for the PV that just finished; flush the
                    # finish + projection of the one before it
                    nonlocal pending_fin
                    yu_rb = norm_head(php, pqc, y_ps, fast=fast)
                    if pending_fin is not None:
                        norm_finish(*pending_fin)
                        if pending_fin[0] == 2:
                            emit_proj(pending_fin[1])
                    pending_fin = (php, pqc, yu_rb)

                last_iter = iters[-1]
                for qc, hp in iters:
                    pts = []
                    sus = list(score_units(qc, hp, pts))
                    if prev is not None:
                        pqc, php, ppts = prev
                        y_ps = psY.tile(
                            [128, 512], F32, tag="y", name=f"y{php}_{pqc}"
                        )
                        pvs = list(pv_units(pqc, php, ppts, y_ps))
                    else:
                        y_ps = None
                        pvs = []
                    # interleave: 1 score pair then ~2 PV matmuls so the
                    # Exp queue on ACT never runs dry during the PV chain
                    nsu, npv = len(sus), len(pvs)
                    pi = 0
                    for si, su in enumerate(sus):
                        su()
                        target = (si + 1) * npv // nsu if nsu else npv
                        while pi < target:
                            pvs[pi]()
                            pi += 1
                    while pi < npv:
                        pvs[pi]()
                        pi += 1
                    if prev is not None:
                        tail_stage(prev[0], prev[1], y_ps, fast=(qc, hp) == last_iter)
                    prev = (qc, hp, pts)
                # drain the pipeline
                pqc, php, ppts = prev
                y_ps = psY.tile([128, 512], F32, tag="y", name=f"y{php}_{pqc}")
                for pv in pv_units(pqc, php, ppts, y_ps):
                    pv()
                tail_stage(pqc, php, y_ps, fast=True)
                norm_finish(*pending_fin)
                emit_proj(NCH - 1)

    _split_waits(nc)
    return nc


_NC_CACHE = {}


def _get_nc():
    if "nc" not in _NC_CACHE:
        _NC_CACHE["nc"] = build_nc()
    return _NC_CACHE["nc"]


def _prep_core_inputs(inputs):
    """Host-side sharding: fold norm/scale/rezero into weights, build the
    per-core input maps."""
    bf16 = ml_dtypes.bfloat16
    x = np.asarray(inputs["x"], np.float32)
    ns_ = np.asarray(inputs["norm_scale"], np.float32)
    nb_ = np.asarray(inputs["norm_bias"], np.float32)
    Wa = np.asarray(inputs["W_attn"], np.float32)
    ba = np.asarray(inputs["b_attn"], np.float32)
    Wp = np.asarray(inputs["W_proj"], np.float32)
    sinks = np.asarray(inputs["sinks"], np.float32)
    rz = float(np.asarray(inputs["rezero"], np.float32).reshape(()))

    C = N_EMBD
    W = (ns_[:, None] * Wa).astype(np.float32)
    beff = (nb_.astype(np.float64) @ Wa.astype(np.float64) + ba).astype(np.float32)
    scale = 1.0 / np.sqrt(np.float32(HD))

    # RoPE tables, interleaved-row layout (64-row periodic)
    freqs = 1.0 / THETA ** (np.arange(0, HD, 2, dtype=np.float64) / HD)  # [32]
    tpos = np.arange(T, dtype=np.float64)
    ang = np.outer(tpos, freqs)  # [T, 32]
    cos_t = np.cos(ang).T  # [32, T]
    sin_t = np.sin(ang).T
    ct = np.empty((128, T), np.float64)
    st = np.empty((128, T), np.float64)
    for blk in range(2):
        r0 = 64 * blk
        ct[r0 + 0 : r0 + 64 : 2] = cos_t
        ct[r0 + 1 : r0 + 64 : 2] = cos_t
        st[r0 + 0 : r0 + 64 : 2] = -sin_t
        st[r0 + 1 : r0 + 64 : 2] = sin_t
    ct = ct.astype(bf16)
    st = st.astype(bf16)

    perm = np.zeros((128, 128), np.float32)
    for i in range(64):
        perm[2 * i, 2 * i + 1] = 1.0
        perm[2 * i + 1, 2 * i] = 1.0
    mir = np.zeros((128, 64), np.float32)
    for i in range(64):
        mir[64 + i, i] = 1.0
    ones = np.ones((1, 512), np.float32)
    swap_idx = np.arange(128)
    swap_idx = swap_idx + 1 - 2 * (swap_idx % 2)  # pairwise swap

    in_maps = []
    for d in range(NCORES):
        b = d // 4
        g = d % 4
        heads = [3 * g + j for j in range(HL)]

        wqk = np.empty((C, 384), np.float32)
        bqk = np.empty((128, 3), np.float32)
        # c0 = [q_h0 | q_h1], c1 = [k_h0 | k_h1], c2 = [q_h2 | k_h2]
        h0, h1, h2 = heads
        wqk[:, 0:64] = W[:, 64 * h0 : 64 * h0 + 64] * scale
        wqk[:, 64:128] = W[:, 64 * h1 : 64 * h1 + 64] * scale
        wqk[:, 128:192] = W[:, C + 64 * h0 : C + 64 * h0 + 64]
        wqk[:, 192:256] = W[:, C + 64 * h1 : C + 64 * h1 + 64]
        wqk[:, 256:320] = W[:, 64 * h2 : 64 * h2 + 64] * scale
        wqk[:, 320:384] = W[:, C + 64 * h2 : C + 64 * h2 + 64]
        bqk[0:64, 0] = beff[64 * h0 : 64 * h0 + 64] * scale
        bqk[64:128, 0] = beff[64 * h1 : 64 * h1 + 64] * scale
        bqk[0:64, 1] = beff[C + 64 * h0 : C + 64 * h0 + 64]
        bqk[64:128, 1] = beff[C + 64 * h1 : C + 64 * h1 + 64]
        bqk[0:64, 2] = beff[64 * h2 : 64 * h2 + 64] * scale
        bqk[64:128, 2] = beff[C + 64 * h2 : C + 64 * h2 + 64]
        bqksw = bqk[swap_idx, :].copy()

        wv = np.zeros((C, VW), np.float32)
        bv = np.zeros((1, VW), np.float32)
        for j, h in enumerate(heads):
            wv[:, 65 * j : 65 * j + 64] = W[:, 2 * C + 64 * h : 2 * C + 64 * h + 64]
            bv[0, 65 * j : 65 * j + 64] = beff[2 * C + 64 * h : 2 * C + 64 * h + 64]
            bv[0, 65 * j + 64] = 1.0

        # projection weights: heads 0,1 packed on partitions; head 2 + pad
        wp = np.zeros((128, 2, C), np.float32)
        wp[0:64, 0, :] = Wp[64 * h0 : 64 * h0 + 64, :] * rz
        wp[64:128, 0, :] = Wp[64 * h1 : 64 * h1 + 64, :] * rz
        wp[0:64, 1, :] = Wp[64 * h2 : 64 * h2 + 64, :] * rz

        sinkrow = np.zeros((1, VW), np.float32)
        for j, h in enumerate(heads):
            sinkrow[0, 65 * j + 64] = np.exp(np.float64(sinks[h]))

        in_maps.append(
            {
                "xt": np.ascontiguousarray(x[b].T).astype(bf16),
                "wqk": wqk.astype(bf16),
                "bqk": bqk,
                "bqksw": bqksw,
                "wv": wv.astype(bf16),
                "bv": bv.astype(bf16),
                "wp": wp.astype(bf16),
                "ct": ct,
                "st": st,
                "perm": perm.astype(bf16),
                "mir": mir.astype(bf16),
                "sinkrow": sinkrow,
                "ones": ones.astype(bf16),
            }
        )

    bias_out = (np.asarray(inputs["b_proj"], np.float32) * rz).astype(np.float32)
    return in_maps, bias_out


def kernel(**inputs):
    global LAST_RESULTS
    nc = _get_nc()
    in_maps, bias_out = _prep_core_inputs(inputs)
    res = None
    last_exc = None
    for attempt in range(3):
        try:
            res = run_bass_kernel_spmd(nc, in_maps, core_ids=list(range(NCORES)))
            break
        except Exception as e:  # transient NRT_EXEC_UNIT_UNRECOVERABLE etc.
            last_exc = e
            import time as _time

            _time.sleep(2.0)
    if res is None:
        raise last_exc
    LAST_RESULTS = res
    y = np.zeros((B, T, N_EMBD), np.float32)
    for d in range(NCORES):
        y[d // 4] += np.asarray(res.results[d]["yp"], dtype=np.float32)
    y += bias_out[None, None, :]
    return y
